# revision 29
# baseline (speedup 1.0000x reference)
"""Trainium2 Bass kernel: multi-head self-attention (B=2, L=2048, D=1024, H=16).

Sharding: 8 NeuronCores = 2 batches x 4 head-groups (4 heads per core).
Each core computes, for its batch and its 4 heads:
  qkv projection -> full attention -> partial out-projection (its heads'
  contribution to out @ w_out).  The host sums the 4 head-group partials per
  batch and adds b_out.

Device dataflow (all layouts chosen so that no on-chip transpose is needed):
  - host passes x^T  [D, L]  (d-major), so d is on SBUF partitions.
  - qkT  = w_qk.T @ x^T      -> [c=512, L]   (Q^T / K^T per head, dk on partitions)
  - V    = x^T.T  @ w_v_aug  -> [L, 260]     (k-major V, plus a ones column per
                                              head that yields the softmax
                                              denominator for free)
  - S^T  = (K^T)T @ Q^T      -> [k, q] tiles (per head; 2 heads packed in the
                                              128-partition dim, contraction 64)
  - P^T  = exp(S^T)          (no max-subtraction: |scores| <= ~10 in f32, safe)
  - O^T_aug psum += V_aug[k,65].T-contract -> [65, q]
              rows 0-63 = unnormalized head output (dv-major), row 64 = sum_k P
  - normalize: broadcast row 64 across partitions with a K=1 PE matmul
    (ones[1,64].T @ row), reciprocal + multiply on DVE -- no DRAM bounces.
  - y    = O^T_cat.T @ w_out_local -> [L, 1024] partial, DMA'd out.

Scheduling: the steady-state attention loop is co-saturated (ACT exp ~1.07us
per k-tile, PE 4 matmuls ~0.85us), so all remaining projection / out-proj
work is diced into sub-us "filler" units and pumped into the per-kt ACT
slack by a credit scheduler instead of running as blocking phases.  All
matmul operands are bf16; accumulation and softmax stay f32 in PSUM.
"""

import collections
import sys

if "/opt/trn_rl_repo" not in sys.path:
    sys.path.insert(0, "/opt/trn_rl_repo")

import ml_dtypes
import numpy as np

import concourse.bass as bass
import concourse.tile as tile
from concourse import mybir
from concourse.bass_utils import run_bass_kernel_spmd
from concourse.vector_clock import ScopedClock

B, L, D, H, DK = 2, 2048, 1024, 16, 64
HG = 4  # heads per core
F32 = mybir.dt.float32
BF16 = mybir.dt.bfloat16
QC = 512  # l/q chunk width
NQ = L // QC  # 4 chunks
LT = L // 128  # 16 l tiles
KO = D // 128  # 8 contraction subtiles
CV = HG * (DK + 1)  # 260: v columns + per-head ones column

def _ensure_axon_hooks():
    """bass_utils imports antenv.axon_hooks when tracing is requested; the
    image's antenv lacks that module.  Register a null hook so a stray
    BASS_TRACE=1 degrades to an untraced run instead of an ImportError
    (test.py replaces this with the real ctypes hook for profiling)."""
    import sys as _sys

    if "antenv.axon_hooks" in _sys.modules:
        return
    try:
        import antenv
    except ImportError:
        return
    import types

    mod = types.ModuleType("antenv.axon_hooks")
    _state = {"h": None}
    mod.set_axon_ntff_profile_hook = lambda h: _state.__setitem__("h", h)
    mod.get_axon_ntff_profile_hook = lambda: _state["h"]
    _sys.modules["antenv.axon_hooks"] = mod
    antenv.axon_hooks = mod


_ensure_axon_hooks()

_PATCHED = False


def _patch_tile_drain():
    """This container's walrus rejects >1 sem wait on a ctrl instruction
    (setupSyncWait: 'Too many sync wait commands').  Tile's end-of-kernel
    drain accumulates one wait per outstanding semaphore; split the extras
    onto dedicated nops (same semantics: SP observes every sem before the
    final all-engine barrier)."""
    global _PATCHED
    if _PATCHED:
        return

    def _drain_and_barrier(self, tick_clock, wait_clock):
        nc = self.nc
        drain_inst = nc.sync.drain()
        wait_clock.add_sem_waits(
            drain_inst.ins, ScopedClock({None: tick_clock.global_clock})
        )
        si = drain_inst.ins.sync_info
        waits = list(si.on_wait or []) if si is not None else []
        if len(waits) > 1:
            si.on_wait = waits[:1]
            for w in waits[1:]:
                nop = nc.sync.nop()
                nsi = nop.ins.sync_info
                if nsi is None:
                    nop.ins.sync_info = mybir.SyncInfo(on_wait=[w], on_update=[])
                else:
                    nsi.on_wait = [w]
        nc.all_engine_barrier()
        popped = nc._tile_sem_poison_stack.pop()
        assert popped is self._sem_poison
        nc.clear_and_free_semaphores(list(self.sems.allocated().values()))
        nc.all_engine_barrier()

    tile.TileContext._drain_and_barrier = _drain_and_barrier
    _PATCHED = True


def _split_excess_waits(nc, max_waits=1):
    """This toolchain's walrus/ISA config allows only one sem wait per
    instruction, but Tile's wait assignment can attach several.  Hoist the
    extras onto same-engine nops immediately before the instruction (AND
    semantics preserved: the engine blocks on each in program order)."""
    for f in nc.m.functions:
        for blk in f.blocks:
            insts = list(blk.instructions)
            out = []
            changed = False
            for inst in insts:
                si = inst.sync_info
                waits = list(si.on_wait) if (si is not None and si.on_wait) else []
                if len(waits) > max_waits:
                    changed = True
                    for w in waits[:-max_waits]:
                        nop = mybir.InstNoOp(
                            name=f"I-wsplit-{nc.next_id()}",
                            engine=inst.engine,
                            ins=[],
                            outs=[],
                            sync_info=mybir.SyncInfo(on_wait=[w], on_update=[]),
                        )
                        nc.register_instruction(nop, overwrite=True)
                        out.append(nop)
                    si.on_wait = waits[-max_waits:]
                out.append(inst)
            if changed:
                blk.instructions = out

def _drop_redundant_ldweights(nc):
    """The walrus here emits a LDWEIGHTS before every matmul.  Where
    consecutive PE matmuls reuse the same stationary operand, the repeated
    loads are no-ops: drop any LDWEIGHTS whose operand AP matches the
    previous one with only MATMULs in between.  Skip any carrying sync."""
    for f in nc.m.functions:
        for blk in f.blocks:
            insts = list(blk.instructions)
            out = []
            prev_key = None
            changed = False
            for inst in insts:
                nm = type(inst).__name__
                if nm == "InstLdweights":
                    si = inst.sync_info
                    has_sync = si is not None and (si.on_wait or si.on_update)
                    key = (repr(inst.ins), getattr(inst, "tile_position", None))
                    if not has_sync and prev_key == key:
                        changed = True
                        continue  # weights already resident
                    prev_key = key if not has_sync else None
                elif nm == "InstMatmult":
                    pass  # matmuls don't disturb loaded weights
                elif inst.engine == mybir.EngineType.PE:
                    prev_key = None  # anything else on PE: be conservative
                out.append(inst)
            if changed:
                blk.instructions = out


def build_nc(zero_bias=True):
    _patch_tile_drain()
    nc = bass.Bass()
    xT = nc.declare_dram_parameter("xT", [D, L], BF16, isOutput=False)
    wqk = nc.declare_dram_parameter("wqk", [D, 512], BF16, isOutput=False)
    bqk = nc.declare_dram_parameter("bqk", [512], F32, isOutput=False)
    wv = nc.declare_dram_parameter("wv", [D, CV], BF16, isOutput=False)
    bv = nc.declare_dram_parameter("bv", [CV], BF16, isOutput=False)
    wout = nc.declare_dram_parameter("wout", [2 * 128, 1024], BF16, isOutput=False)
    ones = nc.declare_dram_parameter("ones", [1, L], BF16, isOutput=False)
    y = nc.declare_dram_parameter("out", [L, D], F32, isOutput=True)

    Ident = mybir.ActivationFunctionType.Identity
    Exp = mybir.ActivationFunctionType.Exp
    ADD = mybir.AluOpType.add
    NV0 = 3  # v tiles accumulated in PSUM while the input stream loads

    with tile.TileContext(nc) as tc:
        with (
            tc.tile_pool(name="per", bufs=1) as per,
            tc.tile_pool(name="xtp", bufs=1) as xtp,
        ):
            wqk_ch = [
                per.tile([128, 512], BF16, tag=f"wqk{o}", name=f"wqk{o}")
                for o in range(KO)
            ]
            wv_sb = per.tile([128, KO, CV], BF16)
            wout_sb = per.tile([128, 2, 1024], BF16)
            bqk_sb = per.tile([128, 4], F32)
            bv_sb = per.tile([1, CV], BF16)
            ones_sb = per.tile([1, L], BF16)
            ones64 = per.tile([128, 64], BF16)  # row 64 used as K=1 bcast stat
            qkT_sb = per.tile([128, 4, L], BF16)
            v_sb = per.tile([128, LT, CV], BF16)
            oT_sb = per.tile([128, 2, L], BF16)
            scratch1 = per.tile([1, 1], F32)
            scratch2 = per.tile([1, 1], F32)
            xT_ch = [
                xtp.tile([128, L], BF16, tag=f"xt{o}", name=f"xt{o}")
                for o in range(KO)
            ]

            # exp table preload with no DMA dependency (memset-sourced)
            nc.vector.memset(ones64[:], 1.0)
            nc.vector.memset(scratch1[:], 0.0)
            nc.scalar.activation(scratch2[:], scratch1[:], Exp)

            # input stream on 4 queues, in consumption order: per d-chunk o
            # the startup o-loop needs wqk[o], xT[o], wv[o] together.
            qs = [nc.sync, nc.gpsimd, nc.scalar]
            qn = [0]

            def dma(out_, in_):
                qs[qn[0] % 3].dma_start(out=out_, in_=in_)
                qn[0] += 1

            nc.scalar.dma_start(out=bqk_sb[:], in_=bqk.rearrange("(s p) -> p s", p=128))
            nc.sync.dma_start(out=ones_sb[:], in_=ones[:])
            for o in range(KO):
                dma(wqk_ch[o][:], wqk[o * 128 : (o + 1) * 128, :])
                dma(xT_ch[o][:], xT[o * 128 : (o + 1) * 128, :])
                dma(wv_sb[:, o, :], wv[o * 128 : (o + 1) * 128, :])
            dma(wout_sb[:, 0, :], wout[0:128, :])
            dma(wout_sb[:, 1, :], wout[128:256, :])
            dma(bv_sb[:], bv[None, :])

            # --- startup projections, o-outer so the PE tracks the DMA
            # stream: slot0 n0 (q cols for item 0), slot1 n0-3 (all K^T for
            # pair 0), and NV0 V tiles accumulate in PSUM as chunks land.
            with (
                tc.tile_pool(name="psB", bufs=1, space="PSUM") as psB,
                tc.tile_pool(name="psC", bufs=1, space="PSUM") as psC,
            ):
                ps_s0 = psB.tile([128, QC], F32, tag="s0n0", name="ps_s0")
                ps_s1 = [
                    psB.tile([128, QC], F32, tag=f"s1n{n}", name=f"ps_s1{n}")
                    for n in range(NQ)
                ]
                ps_v = [
                    psC.tile([128, CV], F32, tag=f"v{j}", name=f"ps_v{j}")
                    for j in range(NV0)
                ]
                for o in range(KO):
                    nc.tensor.matmul(
                        ps_s0[:],
                        wqk_ch[o][:, 0:128],
                        xT_ch[o][:, 0:QC],
                        start=(o == 0),
                        stop=(o == KO - 1),
                    )
                    for n in range(NQ):
                        nc.tensor.matmul(
                            ps_s1[n][:],
                            wqk_ch[o][:, 128:256],
                            xT_ch[o][:, n * QC : (n + 1) * QC],
                            start=(o == 0),
                            stop=(o == KO - 1),
                        )
                    for j in range(NV0):
                        nc.tensor.matmul(
                            ps_v[j][:],
                            xT_ch[o][:, j * 128 : (j + 1) * 128],
                            wv_sb[:, o, :],
                            start=(o == 0),
                            stop=(zero_bias and o == KO - 1),
                        )
                # copybacks; the first two gate st_pair(kt0), so they go
                # first and split across ACT and DVE.
                nc.scalar.activation(
                    qkT_sb[:, 0, 0:QC], ps_s0[:], Ident,
                    bias=bqk_sb[:, 0:1], scale=1.0,
                )
                nc.vector.tensor_scalar(
                    out=qkT_sb[:, 1, 0:QC], in0=ps_s1[0][:],
                    scalar1=bqk_sb[:, 1:2], scalar2=None, op0=ADD,
                )
                for n in range(1, NQ):
                    if n % 2:
                        nc.scalar.activation(
                            qkT_sb[:, 1, n * QC : (n + 1) * QC], ps_s1[n][:],
                            Ident, bias=bqk_sb[:, 1:2], scale=1.0,
                        )
                    else:
                        nc.vector.tensor_scalar(
                            out=qkT_sb[:, 1, n * QC : (n + 1) * QC],
                            in0=ps_s1[n][:],
                            scalar1=bqk_sb[:, 1:2], scalar2=None, op0=ADD,
                        )
                for j in range(NV0):
                    if not zero_bias:
                        nc.tensor.matmul(
                            ps_v[j][:], ones_sb[0:1, 0:128], bv_sb[0:1, :],
                            start=False, stop=True,
                        )
                    nc.vector.tensor_copy(out=v_sb[:, j, :], in_=ps_v[j][:])
                    if zero_bias:
                        nc.vector.memset(v_sb[:, j, 64 :: DK + 1], 1.0)

            # --- attention: 8 items (pair, q-chunk); per k-tile the ACT exp
            # (~1.08us) bounds the loop, PE slack absorbs filler units.
            items = [(p2, qc) for p2 in range(2) for qc in range(NQ)]
            with (
                tc.tile_pool(name="pt", bufs=4) as ptp,
                tc.tile_pool(name="rcp", bufs=3) as rcp,
                tc.tile_pool(name="psST", bufs=2, space="PSUM") as psST,
                tc.tile_pool(name="psOT", bufs=1, space="PSUM") as psOT,
                tc.tile_pool(name="psL", bufs=2, space="PSUM") as psL,
                tc.tile_pool(name="ysb", bufs=3) as ysb,
            ):

                def emit_v(lt):
                    """V_aug k-tile lt = x @ w_v_aug (ones cols by memset when
                    biases are zero).  ~0.87us of PE."""
                    ps = psL.tile([128, CV], F32, tag="px", name="psv")
                    for o in range(KO):
                        nc.tensor.matmul(
                            ps[:],
                            xT_ch[o][:, lt * 128 : (lt + 1) * 128],
                            wv_sb[:, o, :],
                            start=(o == 0),
                            stop=(zero_bias and o == KO - 1),
                        )
                    if not zero_bias:
                        nc.tensor.matmul(
                            ps[:], ones_sb[0:1, 0:128], bv_sb[0:1, :],
                            start=False, stop=True,
                        )
                    nc.vector.tensor_copy(out=v_sb[:, lt, :], in_=ps[:])
                    if zero_bias:
                        nc.vector.memset(v_sb[:, lt, 64 :: DK + 1], 1.0)

                def emit_qk_chunk(s, n):
                    """One n-chunk (512 q cols) of qkT slot s.  8 matmuls of
                    512 whose per-o LDWEIGHTS hide under the previous matmul
                    (256-col splits expose every load).  ~1.76us of PE."""
                    c0 = n * QC
                    ps = psL.tile([128, QC], F32, tag="px", name="psqk")
                    for o in range(KO):
                        nc.tensor.matmul(
                            ps[:],
                            wqk_ch[o][:, s * 128 : (s + 1) * 128],
                            xT_ch[o][:, c0 : c0 + QC],
                            start=(o == 0),
                            stop=(o == KO - 1),
                        )
                    nc.vector.tensor_scalar(
                        out=qkT_sb[:, s, c0 : c0 + QC], in0=ps[:],
                        scalar1=bqk_sb[:, s : s + 1], scalar2=None, op0=ADD,
                    )

                def emit_out_unit(qc, ltl, n2, last=False):
                    """One [128l, 512] block of the out-projection: both
                    pair-halves accumulate in one PSUM tile, then copy + DMA.
                    ~0.45us of PE."""
                    lt = 4 * qc + ltl
                    ps = psL.tile([128, QC], F32, tag="px", name="psy")
                    for s in range(2):
                        nc.tensor.matmul(
                            ps[:],
                            oT_sb[:, s, lt * 128 : (lt + 1) * 128],
                            wout_sb[:, s, n2 * QC : (n2 + 1) * QC],
                            start=(s == 0),
                            stop=(s == 1),
                        )
                    yt = ysb.tile([128, QC], F32, tag="yt", name="yt")
                    if last:
                        nc.scalar.copy(out=yt[:], in_=ps[:])
                    else:
                        nc.vector.tensor_copy(out=yt[:], in_=ps[:])
                    [nc.gpsimd, nc.sync][(ltl + n2) % 2].dma_start(
                        out=y[lt * 128 : (lt + 1) * 128, n2 * QC : (n2 + 1) * QC],
                        in_=yt[:],
                    )

                def norm_copy(po):
                    """Stage O^T_aug to SBUF (frees the PSUM accumulator for
                    the next item) and compute the reciprocal of the P-rowsum
                    row on the DVE queue, async to PE.  The row transposes
                    into 32 partitions (16 elems/lane) so the iterative
                    reciprocal costs ~16 not ~512 lane-cycles, then
                    transposes back: tr2 row 65 = 1/rowsum."""
                    po_sb = rcp.tile([96, QC], F32, tag="po_sb", name="po_sb", bufs=4)
                    nc.vector.tensor_copy(out=po_sb[0:65, :], in_=po[:])
                    tr = rcp.tile([96, QC], F32, tag="tr", name="tr", bufs=2)
                    trR = rcp.tile([96, QC], F32, tag="trR", name="trR", bufs=2)
                    tr2 = rcp.tile([96, QC], F32, tag="tr2", name="tr2", bufs=2)
                    rrb = rcp.tile([96, QC], BF16, tag="rrb", name="rrb", bufs=2)
                    nc.vector.transpose(out=tr[64:96, :], in_=po_sb[64:96, :])
                    nc.vector.reciprocal(
                        out=trR[64:96, 0 :: 32], in_=tr[64:96, 0 :: 32]
                    )
                    nc.vector.transpose(out=tr2[64:96, :], in_=trR[64:96, :])
                    # bf16 row so the K=1 broadcast matmul streams at full
                    # rate (f32 moving data runs the PE at half speed)
                    nc.vector.tensor_copy(out=rrb[64:65, :], in_=tr2[64:65, :])
                    return po_sb, rrb

                def norm_finish(po_sb, rr, he, p2, qc, rb_pool=None, rb_tag="px"):
                    """Broadcast the reciprocal row across 64 partitions with
                    a K=1 matmul (ones.T @ row), then multiply.  PE cost
                    ~0.39us; runs as a filler inside the next item."""
                    pool = rb_pool if rb_pool is not None else psL
                    rbps = pool.tile([64, QC], F32, tag=rb_tag, name="rbps")
                    nc.tensor.matmul(
                        rbps[:], ones64[64:65, :], rr[64:65, :],
                        start=True, stop=True,
                    )
                    nc.vector.tensor_mul(
                        out=oT_sb[he * 64 : (he + 1) * 64, p2, qc * QC : (qc + 1) * QC],
                        in0=po_sb[0:64, :],
                        in1=rbps[:],
                    )

                def st_pair(sq, sk, qc, kt):
                    """S^T for both heads of the pair, written into the two
                    halves of one 2-bank PSUM tile so a single wide ACTIVATE
                    exps both."""
                    ps2 = psST.tile([128, 2, QC], F32, tag="st2", name="st2")
                    nc.tensor.matmul(
                        ps2[:, 0, :],
                        qkT_sb[0:64, sk, kt * 128 : (kt + 1) * 128],
                        qkT_sb[0:64, sq, qc * QC : (qc + 1) * QC],
                        start=True,
                        stop=True,
                    )
                    nc.tensor.matmul(
                        ps2[:, 1, :],
                        qkT_sb[64:128, sk, kt * 128 : (kt + 1) * 128],
                        qkT_sb[64:128, sq, qc * QC : (qc + 1) * QC],
                        start=True,
                        stop=True,
                    )
                    return ps2

                # filler queue: (cost_ns, force_at_gkt_or_None, fn).
                # deadlines: qkT q-cols for item i are first read by the sts
                # prefetch at gkt = 16*i - 1; k-cols n by st(kt=4n) at
                # gkt = base + 4n - 1.
                fillers = collections.deque()

                def add_qk(s, n, g):
                    fillers.append(
                        (1760.0, g, (lambda s=s, n=n: emit_qk_chunk(s, n)))
                    )

                add_qk(0, 1, 15)
                add_qk(0, 2, 31)
                add_qk(0, 3, 47)
                add_qk(3, 0, 63)
                add_qk(2, 0, 63)
                add_qk(3, 1, 66)
                add_qk(3, 2, 70)
                add_qk(3, 3, 74)
                add_qk(2, 1, 79)
                add_qk(2, 2, 95)
                add_qk(2, 3, 111)

                state = {"credit": 0.0, "v_done": NV0, "st_next": 0}

                # prefetch queue of S^T tiles in global kt order; pump() tops
                # it up to depth 2 before a big filler lump so the exp stream
                # never starves behind the lump.
                st_q = collections.deque()

                def st_push():
                    g = state["st_next"]
                    if g >= len(items) * LT:
                        return
                    i2, k2 = divmod(g, LT)
                    pp, _qq = items[i2]
                    st_q.append(st_pair(2 * pp, 2 * pp + 1, _qq, k2))
                    state["st_next"] = g + 1

                def pump(gkt):
                    c = state["credit"]
                    while fillers and fillers[0][1] is not None and fillers[0][1] <= gkt + 2:
                        cost, _, fn = fillers.popleft()
                        if cost >= 900.0 and len(st_q) < 2:
                            st_push()
                        fn()
                        c -= cost
                    c = max(c, -2000.0)
                    # at most ~one filler's worth per kt keeps credit-driven
                    # pops from bunching into multi-us lumps at item ends
                    budget = 900.0
                    while fillers and c >= fillers[0][0] and budget > 0.0:
                        cost, _, fn = fillers.popleft()
                        if cost >= 900.0 and len(st_q) < 2:
                            st_push()
                        fn()
                        c -= cost
                        budget -= cost
                    state["credit"] = min(c, 2000.0)

                SLACK = 210.0
                st_push()
                for idx, (p2, qc) in enumerate(items):
                    po_e = psOT.tile([65, QC], F32, tag="ote", name="ote")
                    po_o = psOT.tile([65, QC], F32, tag="oto", name="oto")
                    for kt in range(LT):
                        gkt = idx * LT + kt
                        ps2 = st_q.popleft()
                        pt2 = ptp.tile([128, 2, QC], BF16, tag="pt2", name="pt2")
                        nc.scalar.activation(pt2[:], ps2[:], Exp)
                        if not st_q:
                            st_push()
                        if idx == 0:
                            # stream remaining V tiles one k-tile ahead
                            while state["v_done"] <= min(kt + 1, LT - 1):
                                emit_v(state["v_done"])
                                state["v_done"] += 1
                        nc.tensor.matmul(
                            po_e[:],
                            v_sb[:, kt, (2 * p2) * 65 : (2 * p2) * 65 + 65],
                            pt2[:, 0, :],
                            start=(kt == 0),
                            stop=(kt == LT - 1),
                        )
                        nc.tensor.matmul(
                            po_o[:],
                            v_sb[:, kt, (2 * p2 + 1) * 65 : (2 * p2 + 1) * 65 + 65],
                            pt2[:, 1, :],
                            start=(kt == 0),
                            stop=(kt == LT - 1),
                        )
                        state["credit"] += SLACK
                        pump(gkt)
                    sb_e, rr_e = norm_copy(po_e)
                    sb_o, rr_o = norm_copy(po_o)
                    if idx + 1 == len(items):
                        # tail: overlap the last normalize's DVE chain with
                        # leftover fillers and the pair-0 halves of the final
                        # out-projection (independent of this normalize), so
                        # the PE stays busy and keeps its clock up.
                        while fillers:
                            _, _, fn = fillers.popleft()
                            fn()
                        held = []
                        for u in range(4):
                            ltl, n2 = divmod(u, 2)
                            lt = 4 * 3 + ltl
                            pool, tag = (psL, "px") if u < 2 else (psST, "st2")
                            ps = pool.tile([128, QC], F32, tag=tag, name=f"tps{u}")
                            nc.tensor.matmul(
                                ps[:],
                                oT_sb[:, 0, lt * 128 : (lt + 1) * 128],
                                wout_sb[:, 0, n2 * QC : (n2 + 1) * QC],
                                start=True,
                                stop=False,
                            )
                            held.append((ps, ltl, n2))
                        norm_finish(sb_e, rr_e, 0, p2, qc, rb_pool=psOT, rb_tag="ote")
                        norm_finish(sb_o, rr_o, 1, p2, qc, rb_pool=psOT, rb_tag="oto")
                        for u, (ps, ltl, n2) in enumerate(held):
                            lt = 4 * 3 + ltl
                            nc.tensor.matmul(
                                ps[:],
                                oT_sb[:, 1, lt * 128 : (lt + 1) * 128],
                                wout_sb[:, 1, n2 * QC : (n2 + 1) * QC],
                                start=False,
                                stop=True,
                            )
                            yt = ysb.tile([128, QC], F32, tag="yt", name="yt")
                            if u % 2 == 1:
                                nc.scalar.copy(out=yt[:], in_=ps[:])
                            else:
                                nc.vector.tensor_copy(out=yt[:], in_=ps[:])
                            [nc.gpsimd, nc.sync][(ltl + n2) % 2].dma_start(
                                out=y[lt * 128 : (lt + 1) * 128, n2 * QC : (n2 + 1) * QC],
                                in_=yt[:],
                            )
                    else:
                        # run the PE half of the normalize as early fillers
                        # inside the next item (the DVE half is already on
                        # the queue), so the PE never blocks on it here.
                        g_norm = (idx + 1) * LT + 10
                        fillers.appendleft(
                            (430.0, g_norm,
                             (lambda sb=sb_o, rr=rr_o, p=p2, q=qc: norm_finish(sb, rr, 1, p, q)))
                        )
                        fillers.appendleft(
                            (430.0, g_norm,
                             (lambda sb=sb_e, rr=rr_e, p=p2, q=qc: norm_finish(sb, rr, 0, p, q)))
                        )
                    if 4 <= idx < 7:
                        oqc = idx - 4
                        for ltl in range(4):
                            for n2 in range(2):
                                fillers.append(
                                    (450.0, (idx + 2) * LT - 10,
                                     (lambda q=oqc, l=ltl, n=n2: emit_out_unit(q, l, n)))
                                )

                # remaining half of the last q-chunk's out-projection
                # (lt 14,15; copybacks alternate DVE/ACT)
                u = 0
                for ltl in range(2, 4):
                    for n2 in range(2):
                        emit_out_unit(3, ltl, n2, last=(u % 2 == 1))
                        u += 1
    _drop_redundant_ldweights(nc)
    _split_excess_waits(nc)
    return nc


def make_in_maps(x, w_qkv, b_qkv, w_out):
    """Per-core input shards.  Core i: batch i//4, head group i%4 (4 heads).

    w_qk column order per core: slots of 128 = (pair0 q | pair0 k | pair1 q |
    pair1 k), each slot = [even head (64) | odd head (64)].  The 1/sqrt(dk)
    scale is folded into the q columns (and q bias entries).
    """
    in_maps = []
    for core in range(8):
        b, g = divmod(core, 4)
        heads = [4 * g + j for j in range(HG)]
        xT = np.ascontiguousarray(x[b].T)
        cols, bias = [], []
        for pair in range(2):
            for qk in range(2):
                for j in range(2):
                    h = heads[2 * pair + j]
                    base = h * 3 * DK + qk * DK
                    c = w_qkv[:, base : base + DK]
                    bb = b_qkv[base : base + DK]
                    if qk == 0:
                        c = c * (1.0 / np.sqrt(DK))
                        bb = bb * (1.0 / np.sqrt(DK))
                    cols.append(c)
                    bias.append(bb)
        wqk = np.ascontiguousarray(np.concatenate(cols, axis=1), dtype=np.float32)
        bqk = np.concatenate(bias).astype(np.float32)
        wv = np.zeros((D, CV), np.float32)
        bv = np.zeros((CV,), np.float32)
        for j, h in enumerate(heads):
            base = h * 3 * DK + 2 * DK
            wv[:, 65 * j : 65 * j + 64] = w_qkv[:, base : base + DK]
            bv[65 * j : 65 * j + 64] = b_qkv[base : base + DK]
            bv[65 * j + 64] = 1.0
        wo = np.ascontiguousarray(w_out[g * 256 : (g + 1) * 256, :], dtype=np.float32)
        bf = ml_dtypes.bfloat16
        in_maps.append(
            {
                "xT": xT.astype(bf),
                "wqk": wqk.astype(bf),
                "bqk": bqk,
                "wv": wv.astype(bf),
                "bv": bv.astype(bf),
                "wout": wo.astype(bf),
                "ones": np.ones((1, L), bf),
            }
        )
    return in_maps


def kernel(**inputs):
    x = np.asarray(inputs["x"], np.float32)
    w_qkv = np.asarray(inputs["w_qkv"], np.float32)
    b_qkv = np.asarray(inputs["b_qkv"], np.float32)
    w_out = np.asarray(inputs["w_out"], np.float32)
    b_out = np.asarray(inputs["b_out"], np.float32)

    in_maps = make_in_maps(x, w_qkv, b_qkv, w_out)
    nc = build_nc(zero_bias=not bool(np.any(b_qkv)))
    res = run_bass_kernel_spmd(nc, in_maps, core_ids=list(range(8)))
    kernel.last_results = res

    out = np.zeros((B, L, D), np.float32)
    for core in range(8):
        out[core // 4] += res.results[core]["out"]
    out += b_out[None, None, :]
    return out


kernel.last_results = None


# revision 32
# speedup vs baseline: 1.0066x; 1.0066x over previous
"""Trainium2 Bass kernel: multi-head self-attention (B=2, L=2048, D=1024, H=16).

Sharding: 8 NeuronCores = 2 batches x 4 head-groups (4 heads per core).
Each core computes, for its batch and its 4 heads:
  qkv projection -> full attention -> partial out-projection (its heads'
  contribution to out @ w_out).  The host sums the 4 head-group partials per
  batch and adds b_out.

Device dataflow (all layouts chosen so that no on-chip transpose is needed):
  - host passes x^T  [D, L]  (d-major), so d is on SBUF partitions.
  - qkT  = w_qk.T @ x^T      -> [c=512, L]   (Q^T / K^T per head, dk on partitions)
  - V    = x^T.T  @ w_v_aug  -> [L, 260]     (k-major V, plus a ones column per
                                              head that yields the softmax
                                              denominator for free)
  - S^T  = (K^T)T @ Q^T      -> [k, q] tiles (per head; 2 heads packed in the
                                              128-partition dim, contraction 64)
  - P^T  = exp(S^T)          (no max-subtraction: |scores| <= ~10 in f32, safe)
  - O^T_aug psum += V_aug[k,65].T-contract -> [65, q]
              rows 0-63 = unnormalized head output (dv-major), row 64 = sum_k P
  - normalize: broadcast row 64 across partitions with a K=1 PE matmul
    (ones[1,64].T @ row), reciprocal + multiply on DVE -- no DRAM bounces.
  - y    = O^T_cat.T @ w_out_local -> [L, 1024] partial, DMA'd out.

Scheduling: the steady-state attention loop is co-saturated (ACT exp ~1.07us
per k-tile, PE 4 matmuls ~0.85us), so all remaining projection / out-proj
work is diced into sub-us "filler" units and pumped into the per-kt ACT
slack by a credit scheduler instead of running as blocking phases.  All
matmul operands are bf16; accumulation and softmax stay f32 in PSUM.
"""

import collections
import sys

if "/opt/trn_rl_repo" not in sys.path:
    sys.path.insert(0, "/opt/trn_rl_repo")

import ml_dtypes
import numpy as np

import concourse.bass as bass
import concourse.tile as tile
from concourse import mybir
from concourse.bass_utils import run_bass_kernel_spmd
from concourse.vector_clock import ScopedClock

B, L, D, H, DK = 2, 2048, 1024, 16, 64
HG = 4  # heads per core
F32 = mybir.dt.float32
BF16 = mybir.dt.bfloat16
QC = 512  # l/q chunk width
NQ = L // QC  # 4 chunks
LT = L // 128  # 16 l tiles
KO = D // 128  # 8 contraction subtiles
CV = HG * (DK + 1)  # 260: v columns + per-head ones column

def _ensure_axon_hooks():
    """bass_utils imports antenv.axon_hooks when tracing is requested; the
    image's antenv lacks that module.  Register a null hook so a stray
    BASS_TRACE=1 degrades to an untraced run instead of an ImportError
    (test.py replaces this with the real ctypes hook for profiling)."""
    import sys as _sys

    if "antenv.axon_hooks" in _sys.modules:
        return
    try:
        import antenv
    except ImportError:
        return
    import types

    mod = types.ModuleType("antenv.axon_hooks")
    _state = {"h": None}
    mod.set_axon_ntff_profile_hook = lambda h: _state.__setitem__("h", h)
    mod.get_axon_ntff_profile_hook = lambda: _state["h"]
    _sys.modules["antenv.axon_hooks"] = mod
    antenv.axon_hooks = mod


_ensure_axon_hooks()

_PATCHED = False


def _patch_tile_drain():
    """This container's walrus rejects >1 sem wait on a ctrl instruction
    (setupSyncWait: 'Too many sync wait commands').  Tile's end-of-kernel
    drain accumulates one wait per outstanding semaphore; split the extras
    onto dedicated nops (same semantics: SP observes every sem before the
    final all-engine barrier)."""
    global _PATCHED
    if _PATCHED:
        return

    def _drain_and_barrier(self, tick_clock, wait_clock):
        nc = self.nc
        drain_inst = nc.sync.drain()
        wait_clock.add_sem_waits(
            drain_inst.ins, ScopedClock({None: tick_clock.global_clock})
        )
        si = drain_inst.ins.sync_info
        waits = list(si.on_wait or []) if si is not None else []
        if len(waits) > 1:
            si.on_wait = waits[:1]
            for w in waits[1:]:
                nop = nc.sync.nop()
                nsi = nop.ins.sync_info
                if nsi is None:
                    nop.ins.sync_info = mybir.SyncInfo(on_wait=[w], on_update=[])
                else:
                    nsi.on_wait = [w]
        nc.all_engine_barrier()
        popped = nc._tile_sem_poison_stack.pop()
        assert popped is self._sem_poison
        nc.clear_and_free_semaphores(list(self.sems.allocated().values()))
        nc.all_engine_barrier()

    tile.TileContext._drain_and_barrier = _drain_and_barrier
    _PATCHED = True


def _split_excess_waits(nc, max_waits=1):
    """This toolchain's walrus/ISA config allows only one sem wait per
    instruction, but Tile's wait assignment can attach several.  Hoist the
    extras onto same-engine nops immediately before the instruction (AND
    semantics preserved: the engine blocks on each in program order)."""
    for f in nc.m.functions:
        for blk in f.blocks:
            insts = list(blk.instructions)
            out = []
            changed = False
            for inst in insts:
                si = inst.sync_info
                waits = list(si.on_wait) if (si is not None and si.on_wait) else []
                if len(waits) > max_waits:
                    changed = True
                    for w in waits[:-max_waits]:
                        nop = mybir.InstNoOp(
                            name=f"I-wsplit-{nc.next_id()}",
                            engine=inst.engine,
                            ins=[],
                            outs=[],
                            sync_info=mybir.SyncInfo(on_wait=[w], on_update=[]),
                        )
                        nc.register_instruction(nop, overwrite=True)
                        out.append(nop)
                    si.on_wait = waits[-max_waits:]
                out.append(inst)
            if changed:
                blk.instructions = out

def _drop_redundant_ldweights(nc):
    """The walrus here emits a LDWEIGHTS before every matmul.  Where
    consecutive PE matmuls reuse the same stationary operand, the repeated
    loads are no-ops: drop any LDWEIGHTS whose operand AP matches the
    previous one with only MATMULs in between.  Skip any carrying sync."""
    for f in nc.m.functions:
        for blk in f.blocks:
            insts = list(blk.instructions)
            out = []
            prev_key = None
            changed = False
            for inst in insts:
                nm = type(inst).__name__
                if nm == "InstLdweights":
                    si = inst.sync_info
                    has_sync = si is not None and (si.on_wait or si.on_update)
                    key = (repr(inst.ins), getattr(inst, "tile_position", None))
                    if not has_sync and prev_key == key:
                        changed = True
                        continue  # weights already resident
                    prev_key = key if not has_sync else None
                elif nm == "InstMatmult":
                    pass  # matmuls don't disturb loaded weights
                elif inst.engine == mybir.EngineType.PE:
                    prev_key = None  # anything else on PE: be conservative
                out.append(inst)
            if changed:
                blk.instructions = out


def build_nc(zero_bias=True):
    _patch_tile_drain()
    nc = bass.Bass()
    xT = nc.declare_dram_parameter("xT", [D, L], BF16, isOutput=False)
    wqk = nc.declare_dram_parameter("wqk", [D, 512], BF16, isOutput=False)
    bqk = nc.declare_dram_parameter("bqk", [512], F32, isOutput=False)
    wv = nc.declare_dram_parameter("wv", [D, CV], BF16, isOutput=False)
    bv = nc.declare_dram_parameter("bv", [CV], BF16, isOutput=False)
    wout = nc.declare_dram_parameter("wout", [2 * 128, 1024], BF16, isOutput=False)
    ones = nc.declare_dram_parameter("ones", [1, L], BF16, isOutput=False)
    y = nc.declare_dram_parameter("out", [L, D], F32, isOutput=True)

    Ident = mybir.ActivationFunctionType.Identity
    Exp = mybir.ActivationFunctionType.Exp
    ADD = mybir.AluOpType.add
    NV0 = 3  # v tiles accumulated in PSUM while the input stream loads

    with tile.TileContext(nc) as tc:
        with (
            tc.tile_pool(name="per", bufs=1) as per,
            tc.tile_pool(name="xtp", bufs=1) as xtp,
        ):
            wqk_ch = [
                per.tile([128, 512], BF16, tag=f"wqk{o}", name=f"wqk{o}")
                for o in range(KO)
            ]
            wv_sb = per.tile([128, KO, CV], BF16)
            wout_sb = per.tile([128, 2, 1024], BF16)
            bqk_sb = per.tile([128, 4], F32)
            bv_sb = per.tile([1, CV], BF16)
            ones_sb = per.tile([1, L], BF16)
            ones64 = per.tile([128, 64], BF16)  # row 64 used as K=1 bcast stat
            qkT_sb = per.tile([128, 4, L], BF16)
            v_sb = per.tile([128, LT, CV], BF16)
            oT_sb = per.tile([128, 2, L], BF16)
            scratch1 = per.tile([1, 1], F32)
            scratch2 = per.tile([1, 1], F32)
            xT_ch = [
                xtp.tile([128, L], BF16, tag=f"xt{o}", name=f"xt{o}")
                for o in range(KO)
            ]

            # exp table preload with no DMA dependency (memset-sourced)
            nc.vector.memset(ones64[:], 1.0)
            nc.vector.memset(scratch1[:], 0.0)
            nc.scalar.activation(scratch2[:], scratch1[:], Exp)

            # input stream on 4 queues, in consumption order: per d-chunk o
            # the startup o-loop needs wqk[o], xT[o], wv[o] together.
            qs = [nc.sync, nc.gpsimd, nc.scalar]
            qn = [0]

            def dma(out_, in_):
                qs[qn[0] % 3].dma_start(out=out_, in_=in_)
                qn[0] += 1

            nc.scalar.dma_start(out=bqk_sb[:], in_=bqk.rearrange("(s p) -> p s", p=128))
            nc.sync.dma_start(out=ones_sb[:], in_=ones[:])
            for o in range(KO):
                # rotate each stream across queues per chunk so the 4MB xT
                # stream isn't serialized behind one DGE (~160GB/s/queue)
                qs[o % 3].dma_start(
                    out=wqk_ch[o][:], in_=wqk[o * 128 : (o + 1) * 128, :]
                )
                qs[(o + 1) % 3].dma_start(
                    out=xT_ch[o][:], in_=xT[o * 128 : (o + 1) * 128, :]
                )
                qs[(o + 2) % 3].dma_start(
                    out=wv_sb[:, o, :], in_=wv[o * 128 : (o + 1) * 128, :]
                )
            dma(wout_sb[:, 0, :], wout[0:128, :])
            dma(wout_sb[:, 1, :], wout[128:256, :])
            dma(bv_sb[:], bv[None, :])

            # --- startup projections, o-outer so the PE tracks the DMA
            # stream: slot0 n0 (q cols for item 0), slot1 n0-3 (all K^T for
            # pair 0), and NV0 V tiles accumulate in PSUM as chunks land.
            with (
                tc.tile_pool(name="psB", bufs=1, space="PSUM") as psB,
                tc.tile_pool(name="psC", bufs=1, space="PSUM") as psC,
            ):
                ps_s0 = psB.tile([128, QC], F32, tag="s0n0", name="ps_s0")
                ps_s1 = [
                    psB.tile([128, QC], F32, tag=f"s1n{n}", name=f"ps_s1{n}")
                    for n in range(NQ)
                ]
                ps_v = [
                    psC.tile([128, CV], F32, tag=f"v{j}", name=f"ps_v{j}")
                    for j in range(NV0)
                ]
                for o in range(KO):
                    nc.tensor.matmul(
                        ps_s0[:],
                        wqk_ch[o][:, 0:128],
                        xT_ch[o][:, 0:QC],
                        start=(o == 0),
                        stop=(o == KO - 1),
                    )
                    for n in range(NQ):
                        nc.tensor.matmul(
                            ps_s1[n][:],
                            wqk_ch[o][:, 128:256],
                            xT_ch[o][:, n * QC : (n + 1) * QC],
                            start=(o == 0),
                            stop=(o == KO - 1),
                        )
                    for j in range(NV0):
                        nc.tensor.matmul(
                            ps_v[j][:],
                            xT_ch[o][:, j * 128 : (j + 1) * 128],
                            wv_sb[:, o, :],
                            start=(o == 0),
                            stop=(zero_bias and o == KO - 1),
                        )
                # copybacks; the first two gate st_pair(kt0), so they go
                # first and split across ACT and DVE.
                nc.scalar.activation(
                    qkT_sb[:, 0, 0:QC], ps_s0[:], Ident,
                    bias=bqk_sb[:, 0:1], scale=1.0,
                )
                nc.vector.tensor_scalar(
                    out=qkT_sb[:, 1, 0:QC], in0=ps_s1[0][:],
                    scalar1=bqk_sb[:, 1:2], scalar2=None, op0=ADD,
                )
                for n in range(1, NQ):
                    if n % 2:
                        nc.scalar.activation(
                            qkT_sb[:, 1, n * QC : (n + 1) * QC], ps_s1[n][:],
                            Ident, bias=bqk_sb[:, 1:2], scale=1.0,
                        )
                    else:
                        nc.vector.tensor_scalar(
                            out=qkT_sb[:, 1, n * QC : (n + 1) * QC],
                            in0=ps_s1[n][:],
                            scalar1=bqk_sb[:, 1:2], scalar2=None, op0=ADD,
                        )
                for j in range(NV0):
                    if not zero_bias:
                        nc.tensor.matmul(
                            ps_v[j][:], ones_sb[0:1, 0:128], bv_sb[0:1, :],
                            start=False, stop=True,
                        )
                    nc.vector.tensor_copy(out=v_sb[:, j, :], in_=ps_v[j][:])
                    if zero_bias:
                        nc.vector.memset(v_sb[:, j, 64 :: DK + 1], 1.0)

            # --- attention: 8 items (pair, q-chunk); per k-tile the ACT exp
            # (~1.08us) bounds the loop, PE slack absorbs filler units.
            items = [(p2, qc) for p2 in range(2) for qc in range(NQ)]
            with (
                tc.tile_pool(name="pt", bufs=4) as ptp,
                tc.tile_pool(name="rcp", bufs=3) as rcp,
                tc.tile_pool(name="psST", bufs=2, space="PSUM") as psST,
                tc.tile_pool(name="psOT", bufs=1, space="PSUM") as psOT,
                tc.tile_pool(name="psL", bufs=2, space="PSUM") as psL,
                tc.tile_pool(name="ysb", bufs=4) as ysb,
            ):

                def emit_v(lt):
                    """V_aug k-tile lt = x @ w_v_aug (ones cols by memset when
                    biases are zero).  ~0.87us of PE."""
                    ps = psL.tile([128, CV], F32, tag="px", name="psv")
                    for o in range(KO):
                        nc.tensor.matmul(
                            ps[:],
                            xT_ch[o][:, lt * 128 : (lt + 1) * 128],
                            wv_sb[:, o, :],
                            start=(o == 0),
                            stop=(zero_bias and o == KO - 1),
                        )
                    if not zero_bias:
                        nc.tensor.matmul(
                            ps[:], ones_sb[0:1, 0:128], bv_sb[0:1, :],
                            start=False, stop=True,
                        )
                    nc.vector.tensor_copy(out=v_sb[:, lt, :], in_=ps[:])
                    if zero_bias:
                        nc.vector.memset(v_sb[:, lt, 64 :: DK + 1], 1.0)

                def emit_qk_chunk(s, n):
                    """One n-chunk (512 q cols) of qkT slot s.  8 matmuls of
                    512 whose per-o LDWEIGHTS hide under the previous matmul
                    (256-col splits expose every load).  ~1.76us of PE."""
                    c0 = n * QC
                    ps = psL.tile([128, QC], F32, tag="px", name="psqk")
                    for o in range(KO):
                        nc.tensor.matmul(
                            ps[:],
                            wqk_ch[o][:, s * 128 : (s + 1) * 128],
                            xT_ch[o][:, c0 : c0 + QC],
                            start=(o == 0),
                            stop=(o == KO - 1),
                        )
                    nc.vector.tensor_scalar(
                        out=qkT_sb[:, s, c0 : c0 + QC], in0=ps[:],
                        scalar1=bqk_sb[:, s : s + 1], scalar2=None, op0=ADD,
                    )

                def emit_out_unit(qc, ltl, n2, last=False):
                    """One [128l, 512] block of the out-projection: both
                    pair-halves accumulate in one PSUM tile, then copy + DMA.
                    ~0.45us of PE."""
                    lt = 4 * qc + ltl
                    ps = psL.tile([128, QC], F32, tag="px", name="psy")
                    for s in range(2):
                        nc.tensor.matmul(
                            ps[:],
                            oT_sb[:, s, lt * 128 : (lt + 1) * 128],
                            wout_sb[:, s, n2 * QC : (n2 + 1) * QC],
                            start=(s == 0),
                            stop=(s == 1),
                        )
                    yt = ysb.tile([128, QC], F32, tag="yt", name="yt")
                    if last:
                        nc.scalar.copy(out=yt[:], in_=ps[:])
                    else:
                        nc.vector.tensor_copy(out=yt[:], in_=ps[:])
                    [nc.gpsimd, nc.sync][(ltl + n2) % 2].dma_start(
                        out=y[lt * 128 : (lt + 1) * 128, n2 * QC : (n2 + 1) * QC],
                        in_=yt[:],
                    )

                def norm_copy(po):
                    """Stage O^T_aug to SBUF (frees the PSUM accumulator for
                    the next item) and compute the reciprocal of the P-rowsum
                    row on the DVE queue, async to PE.  The row transposes
                    into 32 partitions (16 elems/lane) so the iterative
                    reciprocal costs ~16 not ~512 lane-cycles, then
                    transposes back: tr2 row 65 = 1/rowsum."""
                    po_sb = rcp.tile([96, QC], F32, tag="po_sb", name="po_sb", bufs=4)
                    nc.vector.tensor_copy(out=po_sb[0:65, :], in_=po[:])
                    tr = rcp.tile([96, QC], F32, tag="tr", name="tr", bufs=2)
                    trR = rcp.tile([96, QC], F32, tag="trR", name="trR", bufs=2)
                    tr2 = rcp.tile([96, QC], F32, tag="tr2", name="tr2", bufs=2)
                    rrb = rcp.tile([96, QC], BF16, tag="rrb", name="rrb", bufs=2)
                    nc.vector.transpose(out=tr[64:96, :], in_=po_sb[64:96, :])
                    nc.vector.reciprocal(
                        out=trR[64:96, 0 :: 32], in_=tr[64:96, 0 :: 32]
                    )
                    nc.vector.transpose(out=tr2[64:96, :], in_=trR[64:96, :])
                    # bf16 row so the K=1 broadcast matmul streams at full
                    # rate (f32 moving data runs the PE at half speed)
                    nc.vector.tensor_copy(out=rrb[64:65, :], in_=tr2[64:65, :])
                    return po_sb, rrb

                def norm_finish(po_sb, rr, he, p2, qc, rb_pool=None, rb_tag="px"):
                    """Broadcast the reciprocal row across 64 partitions with
                    a K=1 matmul (ones.T @ row), then multiply.  PE cost
                    ~0.39us; runs as a filler inside the next item."""
                    pool = rb_pool if rb_pool is not None else psL
                    rbps = pool.tile([64, QC], F32, tag=rb_tag, name="rbps")
                    nc.tensor.matmul(
                        rbps[:], ones64[64:65, :], rr[64:65, :],
                        start=True, stop=True,
                    )
                    nc.vector.tensor_mul(
                        out=oT_sb[he * 64 : (he + 1) * 64, p2, qc * QC : (qc + 1) * QC],
                        in0=po_sb[0:64, :],
                        in1=rbps[:],
                    )

                def st_pair(sq, sk, qc, kt):
                    """S^T for both heads of the pair, written into the two
                    halves of one 2-bank PSUM tile so a single wide ACTIVATE
                    exps both."""
                    ps2 = psST.tile([128, 2, QC], F32, tag="st2", name="st2")
                    nc.tensor.matmul(
                        ps2[:, 0, :],
                        qkT_sb[0:64, sk, kt * 128 : (kt + 1) * 128],
                        qkT_sb[0:64, sq, qc * QC : (qc + 1) * QC],
                        start=True,
                        stop=True,
                    )
                    nc.tensor.matmul(
                        ps2[:, 1, :],
                        qkT_sb[64:128, sk, kt * 128 : (kt + 1) * 128],
                        qkT_sb[64:128, sq, qc * QC : (qc + 1) * QC],
                        start=True,
                        stop=True,
                    )
                    return ps2

                # filler queue: (cost_ns, force_at_gkt_or_None, fn).
                # deadlines: qkT q-cols for item i are first read by the sts
                # prefetch at gkt = 16*i - 1; k-cols n by st(kt=4n) at
                # gkt = base + 4n - 1.
                fillers = collections.deque()

                def add_qk(s, n, g):
                    fillers.append(
                        (1760.0, g, (lambda s=s, n=n: emit_qk_chunk(s, n)))
                    )

                add_qk(0, 1, 15)
                add_qk(0, 2, 31)
                add_qk(0, 3, 47)
                add_qk(3, 0, 63)
                add_qk(2, 0, 63)
                add_qk(3, 1, 66)
                add_qk(3, 2, 70)
                add_qk(3, 3, 74)
                add_qk(2, 1, 79)
                add_qk(2, 2, 95)
                add_qk(2, 3, 111)

                state = {"credit": 0.0, "v_done": NV0, "st_next": 0}

                # prefetch queue of S^T tiles in global kt order; pump() tops
                # it up to depth 2 before a big filler lump so the exp stream
                # never starves behind the lump.
                st_q = collections.deque()

                def st_push():
                    g = state["st_next"]
                    if g >= len(items) * LT:
                        return
                    i2, k2 = divmod(g, LT)
                    pp, _qq = items[i2]
                    st_q.append(st_pair(2 * pp, 2 * pp + 1, _qq, k2))
                    state["st_next"] = g + 1

                def pump(gkt):
                    c = state["credit"]
                    while fillers and fillers[0][1] is not None and fillers[0][1] <= gkt + 2:
                        cost, _, fn = fillers.popleft()
                        if cost >= 900.0 and len(st_q) < 2:
                            st_push()
                        fn()
                        c -= cost
                    c = max(c, -2000.0)
                    # at most ~one filler's worth per kt keeps credit-driven
                    # pops from bunching into multi-us lumps at item ends
                    budget = 900.0
                    while fillers and c >= fillers[0][0] and budget > 0.0:
                        cost, _, fn = fillers.popleft()
                        if cost >= 900.0 and len(st_q) < 2:
                            st_push()
                        fn()
                        c -= cost
                        budget -= cost
                    state["credit"] = min(c, 2000.0)

                SLACK = 210.0
                st_push()
                for idx, (p2, qc) in enumerate(items):
                    po_e = psOT.tile([65, QC], F32, tag="ote", name="ote")
                    po_o = psOT.tile([65, QC], F32, tag="oto", name="oto")
                    for kt in range(LT):
                        gkt = idx * LT + kt
                        ps2 = st_q.popleft()
                        pt2 = ptp.tile([128, 2, QC], BF16, tag="pt2", name="pt2")
                        nc.scalar.activation(pt2[:], ps2[:], Exp)
                        if not st_q:
                            st_push()
                        if idx == 0:
                            # stream remaining V tiles one k-tile ahead
                            while state["v_done"] <= min(kt + 1, LT - 1):
                                emit_v(state["v_done"])
                                state["v_done"] += 1
                        nc.tensor.matmul(
                            po_e[:],
                            v_sb[:, kt, (2 * p2) * 65 : (2 * p2) * 65 + 65],
                            pt2[:, 0, :],
                            start=(kt == 0),
                            stop=(kt == LT - 1),
                        )
                        nc.tensor.matmul(
                            po_o[:],
                            v_sb[:, kt, (2 * p2 + 1) * 65 : (2 * p2 + 1) * 65 + 65],
                            pt2[:, 1, :],
                            start=(kt == 0),
                            stop=(kt == LT - 1),
                        )
                        state["credit"] += SLACK
                        pump(gkt)
                    sb_e, rr_e = norm_copy(po_e)
                    sb_o, rr_o = norm_copy(po_o)
                    if idx + 1 == len(items):
                        # tail: overlap the last normalize's DVE chain with
                        # leftover fillers and the pair-0 halves of the final
                        # out-projection (independent of this normalize), so
                        # the PE stays busy and keeps its clock up.
                        while fillers:
                            _, _, fn = fillers.popleft()
                            fn()
                        held = []
                        for u in range(4):
                            ltl, n2 = divmod(u, 2)
                            lt = 4 * 3 + ltl
                            pool, tag = (psL, "px") if u < 2 else (psST, "st2")
                            ps = pool.tile([128, QC], F32, tag=tag, name=f"tps{u}")
                            nc.tensor.matmul(
                                ps[:],
                                oT_sb[:, 0, lt * 128 : (lt + 1) * 128],
                                wout_sb[:, 0, n2 * QC : (n2 + 1) * QC],
                                start=True,
                                stop=False,
                            )
                            held.append((ps, ltl, n2))
                        norm_finish(sb_e, rr_e, 0, p2, qc, rb_pool=psOT, rb_tag="ote")
                        norm_finish(sb_o, rr_o, 1, p2, qc, rb_pool=psOT, rb_tag="oto")
                        for u, (ps, ltl, n2) in enumerate(held):
                            lt = 4 * 3 + ltl
                            nc.tensor.matmul(
                                ps[:],
                                oT_sb[:, 1, lt * 128 : (lt + 1) * 128],
                                wout_sb[:, 1, n2 * QC : (n2 + 1) * QC],
                                start=False,
                                stop=True,
                            )
                            yt = ysb.tile([128, QC], F32, tag="yt", name="yt")
                            if u % 2 == 1:
                                nc.scalar.copy(out=yt[:], in_=ps[:])
                            else:
                                nc.vector.tensor_copy(out=yt[:], in_=ps[:])
                            [nc.gpsimd, nc.sync][(ltl + n2) % 2].dma_start(
                                out=y[lt * 128 : (lt + 1) * 128, n2 * QC : (n2 + 1) * QC],
                                in_=yt[:],
                            )
                    else:
                        # run the PE half of the normalize as early fillers
                        # inside the next item (the DVE half is already on
                        # the queue), so the PE never blocks on it here.
                        g_norm = (idx + 1) * LT + 10
                        fillers.appendleft(
                            (430.0, g_norm,
                             (lambda sb=sb_o, rr=rr_o, p=p2, q=qc: norm_finish(sb, rr, 1, p, q)))
                        )
                        fillers.appendleft(
                            (430.0, g_norm,
                             (lambda sb=sb_e, rr=rr_e, p=p2, q=qc: norm_finish(sb, rr, 0, p, q)))
                        )
                    if 4 <= idx < 7:
                        oqc = idx - 4
                        for ltl in range(4):
                            for n2 in range(2):
                                # the last two qc2 units deliberately have no
                                # deadline: they surface in the tail drain,
                                # giving the PE real work (whose copybacks
                                # queue behind the normalize's DVE chain)
                                # while the final normalize runs.
                                g_emit = (idx + 2) * LT - 10
                                if oqc == 2 and ltl == 3:
                                    g_emit = None
                                fillers.append(
                                    (450.0, g_emit,
                                     (lambda q=oqc, l=ltl, n=n2: emit_out_unit(q, l, n)))
                                )

                # remaining half of the last q-chunk's out-projection
                # (lt 14,15; copybacks alternate DVE/ACT)
                u = 0
                for ltl in range(2, 4):
                    for n2 in range(2):
                        emit_out_unit(3, ltl, n2, last=(u % 2 == 1))
                        u += 1
    _drop_redundant_ldweights(nc)
    _split_excess_waits(nc)
    return nc


def make_in_maps(x, w_qkv, b_qkv, w_out):
    """Per-core input shards.  Core i: batch i//4, head group i%4 (4 heads).

    w_qk column order per core: slots of 128 = (pair0 q | pair0 k | pair1 q |
    pair1 k), each slot = [even head (64) | odd head (64)].  The 1/sqrt(dk)
    scale is folded into the q columns (and q bias entries).
    """
    in_maps = []
    for core in range(8):
        b, g = divmod(core, 4)
        heads = [4 * g + j for j in range(HG)]
        xT = np.ascontiguousarray(x[b].T)
        cols, bias = [], []
        for pair in range(2):
            for qk in range(2):
                for j in range(2):
                    h = heads[2 * pair + j]
                    base = h * 3 * DK + qk * DK
                    c = w_qkv[:, base : base + DK]
                    bb = b_qkv[base : base + DK]
                    if qk == 0:
                        c = c * (1.0 / np.sqrt(DK))
                        bb = bb * (1.0 / np.sqrt(DK))
                    cols.append(c)
                    bias.append(bb)
        wqk = np.ascontiguousarray(np.concatenate(cols, axis=1), dtype=np.float32)
        bqk = np.concatenate(bias).astype(np.float32)
        wv = np.zeros((D, CV), np.float32)
        bv = np.zeros((CV,), np.float32)
        for j, h in enumerate(heads):
            base = h * 3 * DK + 2 * DK
            wv[:, 65 * j : 65 * j + 64] = w_qkv[:, base : base + DK]
            bv[65 * j : 65 * j + 64] = b_qkv[base : base + DK]
            bv[65 * j + 64] = 1.0
        wo = np.ascontiguousarray(w_out[g * 256 : (g + 1) * 256, :], dtype=np.float32)
        bf = ml_dtypes.bfloat16
        in_maps.append(
            {
                "xT": xT.astype(bf),
                "wqk": wqk.astype(bf),
                "bqk": bqk,
                "wv": wv.astype(bf),
                "bv": bv.astype(bf),
                "wout": wo.astype(bf),
                "ones": np.ones((1, L), bf),
            }
        )
    return in_maps


def kernel(**inputs):
    x = np.asarray(inputs["x"], np.float32)
    w_qkv = np.asarray(inputs["w_qkv"], np.float32)
    b_qkv = np.asarray(inputs["b_qkv"], np.float32)
    w_out = np.asarray(inputs["w_out"], np.float32)
    b_out = np.asarray(inputs["b_out"], np.float32)

    in_maps = make_in_maps(x, w_qkv, b_qkv, w_out)
    nc = build_nc(zero_bias=not bool(np.any(b_qkv)))
    res = run_bass_kernel_spmd(nc, in_maps, core_ids=list(range(8)))
    kernel.last_results = res

    out = np.zeros((B, L, D), np.float32)
    for core in range(8):
        out[core // 4] += res.results[core]["out"]
    out += b_out[None, None, :]
    return out


kernel.last_results = None


# revision 34
# speedup vs baseline: 1.0181x; 1.0115x over previous
"""Trainium2 Bass kernel: multi-head self-attention (B=2, L=2048, D=1024, H=16).

Sharding: 8 NeuronCores = 2 batches x 4 head-groups (4 heads per core).
Each core computes, for its batch and its 4 heads:
  qkv projection -> full attention -> partial out-projection (its heads'
  contribution to out @ w_out).  The host sums the 4 head-group partials per
  batch and adds b_out.

Device dataflow (all layouts chosen so that no on-chip transpose is needed):
  - host passes x^T  [D, L]  (d-major), so d is on SBUF partitions.
  - qkT  = w_qk.T @ x^T      -> [c=512, L]   (Q^T / K^T per head, dk on partitions)
  - V    = x^T.T  @ w_v_aug  -> [L, 260]     (k-major V, plus a ones column per
                                              head that yields the softmax
                                              denominator for free)
  - S^T  = (K^T)T @ Q^T      -> [k, q] tiles (per head; 2 heads packed in the
                                              128-partition dim, contraction 64)
  - P^T  = exp(S^T)          (no max-subtraction: |scores| <= ~10 in f32, safe)
  - O^T_aug psum += V_aug[k,65].T-contract -> [65, q]
              rows 0-63 = unnormalized head output (dv-major), row 64 = sum_k P
  - normalize: broadcast row 64 across partitions with a K=1 PE matmul
    (ones[1,64].T @ row), reciprocal + multiply on DVE -- no DRAM bounces.
  - y    = O^T_cat.T @ w_out_local -> [L, 1024] partial, DMA'd out.

Scheduling: the steady-state attention loop is co-saturated (ACT exp ~1.07us
per k-tile, PE 4 matmuls ~0.85us), so all remaining projection / out-proj
work is diced into sub-us "filler" units and pumped into the per-kt ACT
slack by a credit scheduler instead of running as blocking phases.  All
matmul operands are bf16; accumulation and softmax stay f32 in PSUM.
"""

import collections
import sys

if "/opt/trn_rl_repo" not in sys.path:
    sys.path.insert(0, "/opt/trn_rl_repo")

import ml_dtypes
import numpy as np

import concourse.bass as bass
import concourse.tile as tile
from concourse import mybir
from concourse.bass_utils import run_bass_kernel_spmd
from concourse.vector_clock import ScopedClock

B, L, D, H, DK = 2, 2048, 1024, 16, 64
HG = 4  # heads per core
F32 = mybir.dt.float32
BF16 = mybir.dt.bfloat16
QC = 512  # l/q chunk width
NQ = L // QC  # 4 chunks
LT = L // 128  # 16 l tiles
KO = D // 128  # 8 contraction subtiles
CV = HG * (DK + 1)  # 260: v columns + per-head ones column

def _ensure_axon_hooks():
    """bass_utils imports antenv.axon_hooks when tracing is requested; the
    image's antenv lacks that module.  Register a null hook so a stray
    BASS_TRACE=1 degrades to an untraced run instead of an ImportError
    (test.py replaces this with the real ctypes hook for profiling)."""
    import sys as _sys

    if "antenv.axon_hooks" in _sys.modules:
        return
    try:
        import antenv
    except ImportError:
        return
    import types

    mod = types.ModuleType("antenv.axon_hooks")
    _state = {"h": None}
    mod.set_axon_ntff_profile_hook = lambda h: _state.__setitem__("h", h)
    mod.get_axon_ntff_profile_hook = lambda: _state["h"]
    _sys.modules["antenv.axon_hooks"] = mod
    antenv.axon_hooks = mod


_ensure_axon_hooks()

_PATCHED = False


def _patch_tile_drain():
    """This container's walrus rejects >1 sem wait on a ctrl instruction
    (setupSyncWait: 'Too many sync wait commands').  Tile's end-of-kernel
    drain accumulates one wait per outstanding semaphore; split the extras
    onto dedicated nops (same semantics: SP observes every sem before the
    final all-engine barrier)."""
    global _PATCHED
    if _PATCHED:
        return

    def _drain_and_barrier(self, tick_clock, wait_clock):
        nc = self.nc
        drain_inst = nc.sync.drain()
        wait_clock.add_sem_waits(
            drain_inst.ins, ScopedClock({None: tick_clock.global_clock})
        )
        si = drain_inst.ins.sync_info
        waits = list(si.on_wait or []) if si is not None else []
        if len(waits) > 1:
            si.on_wait = waits[:1]
            for w in waits[1:]:
                nop = nc.sync.nop()
                nsi = nop.ins.sync_info
                if nsi is None:
                    nop.ins.sync_info = mybir.SyncInfo(on_wait=[w], on_update=[])
                else:
                    nsi.on_wait = [w]
        nc.all_engine_barrier()
        popped = nc._tile_sem_poison_stack.pop()
        assert popped is self._sem_poison
        nc.clear_and_free_semaphores(list(self.sems.allocated().values()))
        nc.all_engine_barrier()

    tile.TileContext._drain_and_barrier = _drain_and_barrier
    _PATCHED = True


def _split_excess_waits(nc, max_waits=1):
    """This toolchain's walrus/ISA config allows only one sem wait per
    instruction, but Tile's wait assignment can attach several.  Hoist the
    extras onto same-engine nops immediately before the instruction (AND
    semantics preserved: the engine blocks on each in program order)."""
    for f in nc.m.functions:
        for blk in f.blocks:
            insts = list(blk.instructions)
            out = []
            changed = False
            for inst in insts:
                si = inst.sync_info
                waits = list(si.on_wait) if (si is not None and si.on_wait) else []
                if len(waits) > max_waits:
                    changed = True
                    for w in waits[:-max_waits]:
                        nop = mybir.InstNoOp(
                            name=f"I-wsplit-{nc.next_id()}",
                            engine=inst.engine,
                            ins=[],
                            outs=[],
                            sync_info=mybir.SyncInfo(on_wait=[w], on_update=[]),
                        )
                        nc.register_instruction(nop, overwrite=True)
                        out.append(nop)
                    si.on_wait = waits[-max_waits:]
                out.append(inst)
            if changed:
                blk.instructions = out

def _drop_redundant_ldweights(nc):
    """The walrus here emits a LDWEIGHTS before every matmul.  Where
    consecutive PE matmuls reuse the same stationary operand, the repeated
    loads are no-ops: drop any LDWEIGHTS whose operand AP matches the
    previous one with only MATMULs in between.  Skip any carrying sync."""
    for f in nc.m.functions:
        for blk in f.blocks:
            insts = list(blk.instructions)
            out = []
            prev_key = None
            changed = False
            for inst in insts:
                nm = type(inst).__name__
                if nm == "InstLdweights":
                    si = inst.sync_info
                    has_sync = si is not None and (si.on_wait or si.on_update)
                    key = (repr(inst.ins), getattr(inst, "tile_position", None))
                    if not has_sync and prev_key == key:
                        changed = True
                        continue  # weights already resident
                    prev_key = key if not has_sync else None
                elif nm == "InstMatmult":
                    pass  # matmuls don't disturb loaded weights
                elif inst.engine == mybir.EngineType.PE:
                    prev_key = None  # anything else on PE: be conservative
                out.append(inst)
            if changed:
                blk.instructions = out


def build_nc(zero_bias=True):
    _patch_tile_drain()
    nc = bass.Bass()
    xT = nc.declare_dram_parameter("xT", [D, L], BF16, isOutput=False)
    wqk = nc.declare_dram_parameter("wqk", [D, 512], BF16, isOutput=False)
    bqk = nc.declare_dram_parameter("bqk", [512], F32, isOutput=False)
    wv = nc.declare_dram_parameter("wv", [D, CV], BF16, isOutput=False)
    bv = nc.declare_dram_parameter("bv", [CV], BF16, isOutput=False)
    wout = nc.declare_dram_parameter("wout", [2 * 128, 1024], BF16, isOutput=False)
    ones = nc.declare_dram_parameter("ones", [1, L], BF16, isOutput=False)
    y = nc.declare_dram_parameter("out", [L, D], F32, isOutput=True)

    Ident = mybir.ActivationFunctionType.Identity
    Exp = mybir.ActivationFunctionType.Exp
    ADD = mybir.AluOpType.add
    NV0 = 3  # v tiles accumulated in PSUM while the input stream loads

    with tile.TileContext(nc) as tc:
        with (
            tc.tile_pool(name="per", bufs=1) as per,
            tc.tile_pool(name="xtp", bufs=1) as xtp,
        ):
            wqk_ch = [
                per.tile([128, 512], BF16, tag=f"wqk{o}", name=f"wqk{o}")
                for o in range(KO)
            ]
            wv_sb = per.tile([128, KO, CV], BF16)
            wout_sb = per.tile([128, 2, 1024], BF16)
            bqk_sb = per.tile([128, 4], F32)
            bv_sb = per.tile([1, CV], BF16)
            ones_sb = per.tile([1, L], BF16)
            ones64 = per.tile([128, 64], BF16)  # row 64 used as K=1 bcast stat
            qkT_sb = per.tile([128, 4, L], BF16)
            v_sb = per.tile([128, LT, CV], BF16)
            oT_sb = per.tile([128, 2, L], BF16)
            scratch1 = per.tile([1, 1], F32)
            scratch2 = per.tile([1, 1], F32)
            xT_ch = [
                xtp.tile([128, L], BF16, tag=f"xt{o}", name=f"xt{o}")
                for o in range(KO)
            ]

            # exp table preload with no DMA dependency (memset-sourced)
            nc.vector.memset(ones64[:], 1.0)
            nc.vector.memset(scratch1[:], 0.0)
            nc.scalar.activation(scratch2[:], scratch1[:], Exp)

            # input stream on 4 queues, in consumption order: per d-chunk o
            # the startup o-loop needs wqk[o], xT[o], wv[o] together.
            qs = [nc.sync, nc.gpsimd, nc.scalar]
            qn = [0]

            def dma(out_, in_):
                qs[qn[0] % 3].dma_start(out=out_, in_=in_)
                qn[0] += 1

            nc.scalar.dma_start(out=bqk_sb[:], in_=bqk.rearrange("(s p) -> p s", p=128))
            nc.sync.dma_start(out=ones_sb[:], in_=ones[:])
            for o in range(KO):
                # rotate each stream across queues per chunk so the 4MB xT
                # stream isn't serialized behind one DGE (~160GB/s/queue)
                qs[o % 3].dma_start(
                    out=wqk_ch[o][:], in_=wqk[o * 128 : (o + 1) * 128, :]
                )
                qs[(o + 1) % 3].dma_start(
                    out=xT_ch[o][:], in_=xT[o * 128 : (o + 1) * 128, :]
                )
                qs[(o + 2) % 3].dma_start(
                    out=wv_sb[:, o, :], in_=wv[o * 128 : (o + 1) * 128, :]
                )
            dma(wout_sb[:, 0, :], wout[0:128, :])
            dma(wout_sb[:, 1, :], wout[128:256, :])
            dma(bv_sb[:], bv[None, :])

            # --- startup projections, o-outer so the PE tracks the DMA
            # stream: slot0 n0 (q cols for item 0), slot1 n0-3 (all K^T for
            # pair 0), and NV0 V tiles accumulate in PSUM as chunks land.
            with (
                tc.tile_pool(name="psB", bufs=1, space="PSUM") as psB,
                tc.tile_pool(name="psC", bufs=1, space="PSUM") as psC,
            ):
                ps_s0 = psB.tile([128, QC], F32, tag="s0n0", name="ps_s0")
                ps_s1 = [
                    psB.tile([128, QC], F32, tag=f"s1n{n}", name=f"ps_s1{n}")
                    for n in range(NQ)
                ]
                ps_v = [
                    psC.tile([128, CV], F32, tag=f"v{j}", name=f"ps_v{j}")
                    for j in range(NV0)
                ]
                def s0_mm(o):
                    nc.tensor.matmul(
                        ps_s0[:],
                        wqk_ch[o][:, 0:128],
                        xT_ch[o][:, 0:QC],
                        start=(o == 0),
                        stop=(o == KO - 1),
                    )

                def s1_mm(n, o):
                    nc.tensor.matmul(
                        ps_s1[n][:],
                        wqk_ch[o][:, 128:256],
                        xT_ch[o][:, n * QC : (n + 1) * QC],
                        start=(o == 0),
                        stop=(o == KO - 1),
                    )

                def v_mm(j, o):
                    nc.tensor.matmul(
                        ps_v[j][:],
                        xT_ch[o][:, j * 128 : (j + 1) * 128],
                        wv_sb[:, o, :],
                        start=(o == 0),
                        stop=(zero_bias and o == KO - 1),
                    )

                def v_fin(j):
                    if not zero_bias:
                        nc.tensor.matmul(
                            ps_v[j][:], ones_sb[0:1, 0:128], bv_sb[0:1, :],
                            start=False, stop=True,
                        )
                    nc.vector.tensor_copy(out=v_sb[:, j, :], in_=ps_v[j][:])
                    if zero_bias:
                        nc.vector.memset(v_sb[:, j, 64 :: DK + 1], 1.0)

                for o in range(KO - 1):
                    s0_mm(o)
                    for n in range(NQ):
                        s1_mm(n, o)
                    for j in range(NV0):
                        v_mm(j, o)
                # last chunk: only the two matmuls that gate the first
                # scores go first, then their copybacks (ACT + DVE in
                # parallel); the rest streams behind the first exp.
                s0_mm(KO - 1)
                s1_mm(0, KO - 1)
                nc.scalar.activation(
                    qkT_sb[:, 0, 0:QC], ps_s0[:], Ident,
                    bias=bqk_sb[:, 0:1], scale=1.0,
                )
                nc.vector.tensor_scalar(
                    out=qkT_sb[:, 1, 0:QC], in0=ps_s1[0][:],
                    scalar1=bqk_sb[:, 1:2], scalar2=None, op0=ADD,
                )
                for j in range(NV0):
                    v_mm(j, KO - 1)
                    v_fin(j)
                for n in range(1, NQ):
                    s1_mm(n, KO - 1)
                    if n % 2:
                        nc.scalar.activation(
                            qkT_sb[:, 1, n * QC : (n + 1) * QC], ps_s1[n][:],
                            Ident, bias=bqk_sb[:, 1:2], scale=1.0,
                        )
                    else:
                        nc.vector.tensor_scalar(
                            out=qkT_sb[:, 1, n * QC : (n + 1) * QC],
                            in0=ps_s1[n][:],
                            scalar1=bqk_sb[:, 1:2], scalar2=None, op0=ADD,
                        )

            # --- attention: 8 items (pair, q-chunk); per k-tile the ACT exp
            # (~1.08us) bounds the loop, PE slack absorbs filler units.
            items = [(p2, qc) for p2 in range(2) for qc in range(NQ)]
            with (
                tc.tile_pool(name="pt", bufs=4) as ptp,
                tc.tile_pool(name="rcp", bufs=3) as rcp,
                tc.tile_pool(name="psST", bufs=2, space="PSUM") as psST,
                tc.tile_pool(name="psOT", bufs=1, space="PSUM") as psOT,
                tc.tile_pool(name="psL", bufs=2, space="PSUM") as psL,
                tc.tile_pool(name="ysb", bufs=4) as ysb,
            ):

                def emit_v(lt):
                    """V_aug k-tile lt = x @ w_v_aug (ones cols by memset when
                    biases are zero).  ~0.87us of PE."""
                    ps = psL.tile([128, CV], F32, tag="px", name="psv")
                    for o in range(KO):
                        nc.tensor.matmul(
                            ps[:],
                            xT_ch[o][:, lt * 128 : (lt + 1) * 128],
                            wv_sb[:, o, :],
                            start=(o == 0),
                            stop=(zero_bias and o == KO - 1),
                        )
                    if not zero_bias:
                        nc.tensor.matmul(
                            ps[:], ones_sb[0:1, 0:128], bv_sb[0:1, :],
                            start=False, stop=True,
                        )
                    nc.vector.tensor_copy(out=v_sb[:, lt, :], in_=ps[:])
                    if zero_bias:
                        nc.vector.memset(v_sb[:, lt, 64 :: DK + 1], 1.0)

                def emit_qk_chunk(s, n):
                    """One n-chunk (512 q cols) of qkT slot s.  8 matmuls of
                    512 whose per-o LDWEIGHTS hide under the previous matmul
                    (256-col splits expose every load).  ~1.76us of PE."""
                    c0 = n * QC
                    ps = psL.tile([128, QC], F32, tag="px", name="psqk")
                    for o in range(KO):
                        nc.tensor.matmul(
                            ps[:],
                            wqk_ch[o][:, s * 128 : (s + 1) * 128],
                            xT_ch[o][:, c0 : c0 + QC],
                            start=(o == 0),
                            stop=(o == KO - 1),
                        )
                    nc.vector.tensor_scalar(
                        out=qkT_sb[:, s, c0 : c0 + QC], in0=ps[:],
                        scalar1=bqk_sb[:, s : s + 1], scalar2=None, op0=ADD,
                    )

                def emit_out_unit(qc, ltl, n2, last=False):
                    """One [128l, 512] block of the out-projection: both
                    pair-halves accumulate in one PSUM tile, then copy + DMA.
                    ~0.45us of PE."""
                    lt = 4 * qc + ltl
                    ps = psL.tile([128, QC], F32, tag="px", name="psy")
                    for s in range(2):
                        nc.tensor.matmul(
                            ps[:],
                            oT_sb[:, s, lt * 128 : (lt + 1) * 128],
                            wout_sb[:, s, n2 * QC : (n2 + 1) * QC],
                            start=(s == 0),
                            stop=(s == 1),
                        )
                    yt = ysb.tile([128, QC], F32, tag="yt", name="yt")
                    if last:
                        nc.scalar.copy(out=yt[:], in_=ps[:])
                    else:
                        nc.vector.tensor_copy(out=yt[:], in_=ps[:])
                    [nc.gpsimd, nc.sync][(ltl + n2) % 2].dma_start(
                        out=y[lt * 128 : (lt + 1) * 128, n2 * QC : (n2 + 1) * QC],
                        in_=yt[:],
                    )

                def norm_copy(po):
                    """Stage O^T_aug to SBUF (frees the PSUM accumulator for
                    the next item) and compute the reciprocal of the P-rowsum
                    row on the DVE queue, async to PE.  The row transposes
                    into 32 partitions (16 elems/lane) so the iterative
                    reciprocal costs ~16 not ~512 lane-cycles, then
                    transposes back: tr2 row 65 = 1/rowsum."""
                    po_sb = rcp.tile([96, QC], F32, tag="po_sb", name="po_sb", bufs=4)
                    nc.vector.tensor_copy(out=po_sb[0:65, :], in_=po[:])
                    tr = rcp.tile([96, QC], F32, tag="tr", name="tr", bufs=2)
                    trR = rcp.tile([96, QC], F32, tag="trR", name="trR", bufs=2)
                    tr2 = rcp.tile([96, QC], F32, tag="tr2", name="tr2", bufs=2)
                    rrb = rcp.tile([96, QC], BF16, tag="rrb", name="rrb", bufs=2)
                    nc.vector.transpose(out=tr[64:96, :], in_=po_sb[64:96, :])
                    nc.vector.reciprocal(
                        out=trR[64:96, 0 :: 32], in_=tr[64:96, 0 :: 32]
                    )
                    nc.vector.transpose(out=tr2[64:96, :], in_=trR[64:96, :])
                    # bf16 row so the K=1 broadcast matmul streams at full
                    # rate (f32 moving data runs the PE at half speed)
                    nc.vector.tensor_copy(out=rrb[64:65, :], in_=tr2[64:65, :])
                    return po_sb, rrb

                def norm_finish(po_sb, rr, he, p2, qc, rb_pool=None, rb_tag="px"):
                    """Broadcast the reciprocal row across 64 partitions with
                    a K=1 matmul (ones.T @ row), then multiply.  PE cost
                    ~0.39us; runs as a filler inside the next item."""
                    pool = rb_pool if rb_pool is not None else psL
                    rbps = pool.tile([64, QC], F32, tag=rb_tag, name="rbps")
                    nc.tensor.matmul(
                        rbps[:], ones64[64:65, :], rr[64:65, :],
                        start=True, stop=True,
                    )
                    nc.vector.tensor_mul(
                        out=oT_sb[he * 64 : (he + 1) * 64, p2, qc * QC : (qc + 1) * QC],
                        in0=po_sb[0:64, :],
                        in1=rbps[:],
                    )

                def st_pair(sq, sk, qc, kt):
                    """S^T for both heads of the pair, written into the two
                    halves of one 2-bank PSUM tile so a single wide ACTIVATE
                    exps both."""
                    ps2 = psST.tile([128, 2, QC], F32, tag="st2", name="st2")
                    nc.tensor.matmul(
                        ps2[:, 0, :],
                        qkT_sb[0:64, sk, kt * 128 : (kt + 1) * 128],
                        qkT_sb[0:64, sq, qc * QC : (qc + 1) * QC],
                        start=True,
                        stop=True,
                    )
                    nc.tensor.matmul(
                        ps2[:, 1, :],
                        qkT_sb[64:128, sk, kt * 128 : (kt + 1) * 128],
                        qkT_sb[64:128, sq, qc * QC : (qc + 1) * QC],
                        start=True,
                        stop=True,
                    )
                    return ps2

                # filler queue: (cost_ns, force_at_gkt_or_None, fn).
                # deadlines: qkT q-cols for item i are first read by the sts
                # prefetch at gkt = 16*i - 1; k-cols n by st(kt=4n) at
                # gkt = base + 4n - 1.
                fillers = collections.deque()

                def add_qk(s, n, g):
                    fillers.append(
                        (1760.0, g, (lambda s=s, n=n: emit_qk_chunk(s, n)))
                    )

                add_qk(0, 1, 15)
                add_qk(0, 2, 31)
                add_qk(0, 3, 47)
                add_qk(3, 0, 63)
                add_qk(2, 0, 63)
                add_qk(3, 1, 66)
                add_qk(3, 2, 70)
                add_qk(3, 3, 74)
                add_qk(2, 1, 79)
                add_qk(2, 2, 95)
                add_qk(2, 3, 111)

                state = {"credit": 0.0, "v_done": NV0, "st_next": 0}

                # prefetch queue of S^T tiles in global kt order; pump() tops
                # it up to depth 2 before a big filler lump so the exp stream
                # never starves behind the lump.
                st_q = collections.deque()

                def st_push():
                    g = state["st_next"]
                    if g >= len(items) * LT:
                        return
                    i2, k2 = divmod(g, LT)
                    pp, _qq = items[i2]
                    st_q.append(st_pair(2 * pp, 2 * pp + 1, _qq, k2))
                    state["st_next"] = g + 1

                def pump(gkt):
                    c = state["credit"]
                    while fillers and fillers[0][1] is not None and fillers[0][1] <= gkt + 2:
                        cost, _, fn = fillers.popleft()
                        if cost >= 900.0 and len(st_q) < 2:
                            st_push()
                        fn()
                        c -= cost
                    c = max(c, -2000.0)
                    # at most ~one filler's worth per kt keeps credit-driven
                    # pops from bunching into multi-us lumps at item ends
                    budget = 900.0
                    while fillers and c >= fillers[0][0] and budget > 0.0:
                        cost, _, fn = fillers.popleft()
                        if cost >= 900.0 and len(st_q) < 2:
                            st_push()
                        fn()
                        c -= cost
                        budget -= cost
                    state["credit"] = min(c, 2000.0)

                SLACK = 210.0
                st_push()
                for idx, (p2, qc) in enumerate(items):
                    po_e = psOT.tile([65, QC], F32, tag="ote", name="ote")
                    po_o = psOT.tile([65, QC], F32, tag="oto", name="oto")
                    for kt in range(LT):
                        gkt = idx * LT + kt
                        ps2 = st_q.popleft()
                        pt2 = ptp.tile([128, 2, QC], BF16, tag="pt2", name="pt2")
                        nc.scalar.activation(pt2[:], ps2[:], Exp)
                        if not st_q:
                            st_push()
                        if idx == 0:
                            # stream remaining V tiles one k-tile ahead
                            while state["v_done"] <= min(kt + 1, LT - 1):
                                emit_v(state["v_done"])
                                state["v_done"] += 1
                        nc.tensor.matmul(
                            po_e[:],
                            v_sb[:, kt, (2 * p2) * 65 : (2 * p2) * 65 + 65],
                            pt2[:, 0, :],
                            start=(kt == 0),
                            stop=(kt == LT - 1),
                        )
                        nc.tensor.matmul(
                            po_o[:],
                            v_sb[:, kt, (2 * p2 + 1) * 65 : (2 * p2 + 1) * 65 + 65],
                            pt2[:, 1, :],
                            start=(kt == 0),
                            stop=(kt == LT - 1),
                        )
                        state["credit"] += SLACK
                        pump(gkt)
                    sb_e, rr_e = norm_copy(po_e)
                    sb_o, rr_o = norm_copy(po_o)
                    if idx + 1 == len(items):
                        # tail: overlap the last normalize's DVE chain with
                        # leftover fillers and the pair-0 halves of the final
                        # out-projection (independent of this normalize), so
                        # the PE stays busy and keeps its clock up.
                        while fillers:
                            _, _, fn = fillers.popleft()
                            fn()
                        held = []
                        for u in range(4):
                            ltl, n2 = divmod(u, 2)
                            lt = 4 * 3 + ltl
                            pool, tag = (psL, "px") if u < 2 else (psST, "st2")
                            ps = pool.tile([128, QC], F32, tag=tag, name=f"tps{u}")
                            nc.tensor.matmul(
                                ps[:],
                                oT_sb[:, 0, lt * 128 : (lt + 1) * 128],
                                wout_sb[:, 0, n2 * QC : (n2 + 1) * QC],
                                start=True,
                                stop=False,
                            )
                            held.append((ps, ltl, n2))
                        norm_finish(sb_e, rr_e, 0, p2, qc, rb_pool=psOT, rb_tag="ote")
                        norm_finish(sb_o, rr_o, 1, p2, qc, rb_pool=psOT, rb_tag="oto")
                        for u, (ps, ltl, n2) in enumerate(held):
                            lt = 4 * 3 + ltl
                            nc.tensor.matmul(
                                ps[:],
                                oT_sb[:, 1, lt * 128 : (lt + 1) * 128],
                                wout_sb[:, 1, n2 * QC : (n2 + 1) * QC],
                                start=False,
                                stop=True,
                            )
                            yt = ysb.tile([128, QC], F32, tag="yt", name="yt")
                            if u % 2 == 1:
                                nc.scalar.copy(out=yt[:], in_=ps[:])
                            else:
                                nc.vector.tensor_copy(out=yt[:], in_=ps[:])
                            [nc.gpsimd, nc.sync][(ltl + n2) % 2].dma_start(
                                out=y[lt * 128 : (lt + 1) * 128, n2 * QC : (n2 + 1) * QC],
                                in_=yt[:],
                            )
                    else:
                        # run the PE half of the normalize as early fillers
                        # inside the next item (the DVE half is already on
                        # the queue), so the PE never blocks on it here.
                        g_norm = (idx + 1) * LT + 10
                        fillers.appendleft(
                            (430.0, g_norm,
                             (lambda sb=sb_o, rr=rr_o, p=p2, q=qc: norm_finish(sb, rr, 1, p, q)))
                        )
                        fillers.appendleft(
                            (430.0, g_norm,
                             (lambda sb=sb_e, rr=rr_e, p=p2, q=qc: norm_finish(sb, rr, 0, p, q)))
                        )
                    if 4 <= idx < 7:
                        oqc = idx - 4
                        for ltl in range(4):
                            for n2 in range(2):
                                # the last two qc2 units deliberately have no
                                # deadline: they surface in the tail drain,
                                # giving the PE real work (whose copybacks
                                # queue behind the normalize's DVE chain)
                                # while the final normalize runs.
                                g_emit = (idx + 2) * LT - 10
                                if oqc == 2 and ltl >= 2:
                                    g_emit = None
                                fillers.append(
                                    (450.0, g_emit,
                                     (lambda q=oqc, l=ltl, n=n2: emit_out_unit(q, l, n)))
                                )

                # remaining half of the last q-chunk's out-projection
                # (lt 14,15; copybacks alternate DVE/ACT)
                u = 0
                for ltl in range(2, 4):
                    for n2 in range(2):
                        emit_out_unit(3, ltl, n2, last=(u % 2 == 1))
                        u += 1
    _drop_redundant_ldweights(nc)
    _split_excess_waits(nc)
    return nc


def make_in_maps(x, w_qkv, b_qkv, w_out):
    """Per-core input shards.  Core i: batch i//4, head group i%4 (4 heads).

    w_qk column order per core: slots of 128 = (pair0 q | pair0 k | pair1 q |
    pair1 k), each slot = [even head (64) | odd head (64)].  The 1/sqrt(dk)
    scale is folded into the q columns (and q bias entries).
    """
    in_maps = []
    for core in range(8):
        b, g = divmod(core, 4)
        heads = [4 * g + j for j in range(HG)]
        xT = np.ascontiguousarray(x[b].T)
        cols, bias = [], []
        for pair in range(2):
            for qk in range(2):
                for j in range(2):
                    h = heads[2 * pair + j]
                    base = h * 3 * DK + qk * DK
                    c = w_qkv[:, base : base + DK]
                    bb = b_qkv[base : base + DK]
                    if qk == 0:
                        c = c * (1.0 / np.sqrt(DK))
                        bb = bb * (1.0 / np.sqrt(DK))
                    cols.append(c)
                    bias.append(bb)
        wqk = np.ascontiguousarray(np.concatenate(cols, axis=1), dtype=np.float32)
        bqk = np.concatenate(bias).astype(np.float32)
        wv = np.zeros((D, CV), np.float32)
        bv = np.zeros((CV,), np.float32)
        for j, h in enumerate(heads):
            base = h * 3 * DK + 2 * DK
            wv[:, 65 * j : 65 * j + 64] = w_qkv[:, base : base + DK]
            bv[65 * j : 65 * j + 64] = b_qkv[base : base + DK]
            bv[65 * j + 64] = 1.0
        wo = np.ascontiguousarray(w_out[g * 256 : (g + 1) * 256, :], dtype=np.float32)
        bf = ml_dtypes.bfloat16
        in_maps.append(
            {
                "xT": xT.astype(bf),
                "wqk": wqk.astype(bf),
                "bqk": bqk,
                "wv": wv.astype(bf),
                "bv": bv.astype(bf),
                "wout": wo.astype(bf),
                "ones": np.ones((1, L), bf),
            }
        )
    return in_maps


def kernel(**inputs):
    x = np.asarray(inputs["x"], np.float32)
    w_qkv = np.asarray(inputs["w_qkv"], np.float32)
    b_qkv = np.asarray(inputs["b_qkv"], np.float32)
    w_out = np.asarray(inputs["w_out"], np.float32)
    b_out = np.asarray(inputs["b_out"], np.float32)

    in_maps = make_in_maps(x, w_qkv, b_qkv, w_out)
    nc = build_nc(zero_bias=not bool(np.any(b_qkv)))
    res = run_bass_kernel_spmd(nc, in_maps, core_ids=list(range(8)))
    kernel.last_results = res

    out = np.zeros((B, L, D), np.float32)
    for core in range(8):
        out[core // 4] += res.results[core]["out"]
    out += b_out[None, None, :]
    return out


kernel.last_results = None


# revision 38
# speedup vs baseline: 1.0207x; 1.0025x over previous
"""Trainium2 Bass kernel: multi-head self-attention (B=2, L=2048, D=1024, H=16).

Sharding: 8 NeuronCores = 2 batches x 4 head-groups (4 heads per core).
Each core computes, for its batch and its 4 heads:
  qkv projection -> full attention -> partial out-projection (its heads'
  contribution to out @ w_out).  The host sums the 4 head-group partials per
  batch and adds b_out.

Device dataflow (all layouts chosen so that no on-chip transpose is needed):
  - host passes x^T  [D, L]  (d-major), so d is on SBUF partitions.
  - qkT  = w_qk.T @ x^T      -> [c=512, L]   (Q^T / K^T per head, dk on partitions)
  - V    = x^T.T  @ w_v_aug  -> [L, 260]     (k-major V, plus a ones column per
                                              head that yields the softmax
                                              denominator for free)
  - S^T  = (K^T)T @ Q^T      -> [k, q] tiles (per head; 2 heads packed in the
                                              128-partition dim, contraction 64)
  - P^T  = exp(S^T)          (no max-subtraction: |scores| <= ~10 in f32, safe)
  - O^T_aug psum += V_aug[k,65].T-contract -> [65, q]
              rows 0-63 = unnormalized head output (dv-major), row 64 = sum_k P
  - normalize: broadcast row 64 across partitions with a K=1 PE matmul
    (ones[1,64].T @ row), reciprocal + multiply on DVE -- no DRAM bounces.
  - y    = O^T_cat.T @ w_out_local -> [L, 1024] partial, DMA'd out.

Scheduling: the steady-state attention loop is co-saturated (ACT exp ~1.07us
per k-tile, PE 4 matmuls ~0.85us), so all remaining projection / out-proj
work is diced into sub-us "filler" units and pumped into the per-kt ACT
slack by a credit scheduler instead of running as blocking phases.  All
matmul operands are bf16; accumulation and softmax stay f32 in PSUM.
"""

import collections
import sys

if "/opt/trn_rl_repo" not in sys.path:
    sys.path.insert(0, "/opt/trn_rl_repo")

import ml_dtypes
import numpy as np

import concourse.bass as bass
import concourse.tile as tile
from concourse import mybir
from concourse.bass_utils import run_bass_kernel_spmd
from concourse.vector_clock import ScopedClock

B, L, D, H, DK = 2, 2048, 1024, 16, 64
HG = 4  # heads per core
F32 = mybir.dt.float32
BF16 = mybir.dt.bfloat16
QC = 512  # l/q chunk width
NQ = L // QC  # 4 chunks
LT = L // 128  # 16 l tiles
KO = D // 128  # 8 contraction subtiles
CV = HG * (DK + 1)  # 260: v columns + per-head ones column

def _ensure_axon_hooks():
    """bass_utils imports antenv.axon_hooks when tracing is requested; the
    image's antenv lacks that module.  Register a null hook so a stray
    BASS_TRACE=1 degrades to an untraced run instead of an ImportError
    (test.py replaces this with the real ctypes hook for profiling)."""
    import sys as _sys

    if "antenv.axon_hooks" in _sys.modules:
        return
    try:
        import antenv
    except ImportError:
        return
    import types

    mod = types.ModuleType("antenv.axon_hooks")
    _state = {"h": None}
    mod.set_axon_ntff_profile_hook = lambda h: _state.__setitem__("h", h)
    mod.get_axon_ntff_profile_hook = lambda: _state["h"]
    _sys.modules["antenv.axon_hooks"] = mod
    antenv.axon_hooks = mod


_ensure_axon_hooks()

_PATCHED = False


def _patch_tile_drain():
    """This container's walrus rejects >1 sem wait on a ctrl instruction
    (setupSyncWait: 'Too many sync wait commands').  Tile's end-of-kernel
    drain accumulates one wait per outstanding semaphore; split the extras
    onto dedicated nops (same semantics: SP observes every sem before the
    final all-engine barrier)."""
    global _PATCHED
    if _PATCHED:
        return

    def _drain_and_barrier(self, tick_clock, wait_clock):
        nc = self.nc
        drain_inst = nc.sync.drain()
        wait_clock.add_sem_waits(
            drain_inst.ins, ScopedClock({None: tick_clock.global_clock})
        )
        si = drain_inst.ins.sync_info
        waits = list(si.on_wait or []) if si is not None else []
        if len(waits) > 1:
            si.on_wait = waits[:1]
            for w in waits[1:]:
                nop = nc.sync.nop()
                nsi = nop.ins.sync_info
                if nsi is None:
                    nop.ins.sync_info = mybir.SyncInfo(on_wait=[w], on_update=[])
                else:
                    nsi.on_wait = [w]
        nc.all_engine_barrier()
        popped = nc._tile_sem_poison_stack.pop()
        assert popped is self._sem_poison
        nc.clear_and_free_semaphores(list(self.sems.allocated().values()))
        nc.all_engine_barrier()

    tile.TileContext._drain_and_barrier = _drain_and_barrier
    _PATCHED = True


def _split_excess_waits(nc, max_waits=1):
    """This toolchain's walrus/ISA config allows only one sem wait per
    instruction, but Tile's wait assignment can attach several.  Hoist the
    extras onto same-engine nops immediately before the instruction (AND
    semantics preserved: the engine blocks on each in program order)."""
    for f in nc.m.functions:
        for blk in f.blocks:
            insts = list(blk.instructions)
            out = []
            changed = False
            for inst in insts:
                si = inst.sync_info
                waits = list(si.on_wait) if (si is not None and si.on_wait) else []
                if len(waits) > max_waits:
                    changed = True
                    for w in waits[:-max_waits]:
                        nop = mybir.InstNoOp(
                            name=f"I-wsplit-{nc.next_id()}",
                            engine=inst.engine,
                            ins=[],
                            outs=[],
                            sync_info=mybir.SyncInfo(on_wait=[w], on_update=[]),
                        )
                        nc.register_instruction(nop, overwrite=True)
                        out.append(nop)
                    si.on_wait = waits[-max_waits:]
                out.append(inst)
            if changed:
                blk.instructions = out

def _drop_redundant_ldweights(nc):
    """The walrus here emits a LDWEIGHTS before every matmul.  Where
    consecutive PE matmuls reuse the same stationary operand, the repeated
    loads are no-ops: drop any LDWEIGHTS whose operand AP matches the
    previous one with only MATMULs in between.  Skip any carrying sync."""
    for f in nc.m.functions:
        for blk in f.blocks:
            insts = list(blk.instructions)
            out = []
            prev_key = None
            changed = False
            for inst in insts:
                nm = type(inst).__name__
                if nm == "InstLdweights":
                    si = inst.sync_info
                    has_sync = si is not None and (si.on_wait or si.on_update)
                    key = (repr(inst.ins), getattr(inst, "tile_position", None))
                    if not has_sync and prev_key == key:
                        changed = True
                        continue  # weights already resident
                    prev_key = key if not has_sync else None
                elif nm == "InstMatmult":
                    pass  # matmuls don't disturb loaded weights
                elif inst.engine == mybir.EngineType.PE:
                    prev_key = None  # anything else on PE: be conservative
                out.append(inst)
            if changed:
                blk.instructions = out


def build_nc(zero_bias=True):
    _patch_tile_drain()
    nc = bass.Bass()
    xT = nc.declare_dram_parameter("xT", [D, L], BF16, isOutput=False)
    wqk = nc.declare_dram_parameter("wqk", [D, 512], BF16, isOutput=False)
    bqk = nc.declare_dram_parameter("bqk", [512], F32, isOutput=False)
    wv = nc.declare_dram_parameter("wv", [D, CV], BF16, isOutput=False)
    bv = nc.declare_dram_parameter("bv", [CV], BF16, isOutput=False)
    wout = nc.declare_dram_parameter("wout", [2 * 128, 1024], BF16, isOutput=False)
    ones = nc.declare_dram_parameter("ones", [1, L], BF16, isOutput=False)
    y = nc.declare_dram_parameter("out", [L, D], F32, isOutput=True)

    Ident = mybir.ActivationFunctionType.Identity
    Exp = mybir.ActivationFunctionType.Exp
    ADD = mybir.AluOpType.add
    NV0 = 3  # v tiles accumulated in PSUM while the input stream loads

    with tile.TileContext(nc) as tc:
        with (
            tc.tile_pool(name="per", bufs=1) as per,
            tc.tile_pool(name="xtp", bufs=1) as xtp,
        ):
            wqk_ch = [
                per.tile([128, 512], BF16, tag=f"wqk{o}", name=f"wqk{o}")
                for o in range(KO)
            ]
            wv_sb = per.tile([128, KO, CV], BF16)
            wout_sb = per.tile([128, 2, 1024], BF16)
            bqk_sb = per.tile([128, 4], F32)
            bv_sb = per.tile([1, CV], BF16)
            ones_sb = per.tile([1, L], BF16)
            ones64 = per.tile([128, 64], BF16)  # row 64 used as K=1 bcast stat
            qkT_sb = per.tile([128, 4, L], BF16)
            v_sb = per.tile([128, LT, CV], BF16)
            oT_sb = per.tile([128, 2, L], BF16)
            scratch1 = per.tile([1, 1], F32)
            scratch2 = per.tile([1, 1], F32)
            xT_ch = [
                xtp.tile([128, L], BF16, tag=f"xt{o}", name=f"xt{o}")
                for o in range(KO)
            ]

            # exp table preload with no DMA dependency (memset-sourced)
            nc.vector.memset(ones64[:], 1.0)
            nc.vector.memset(scratch1[:], 0.0)
            nc.scalar.activation(scratch2[:], scratch1[:], Exp)

            # input stream on 4 queues, in consumption order: per d-chunk o
            # the startup o-loop needs wqk[o], xT[o], wv[o] together.
            qs = [nc.sync, nc.gpsimd, nc.scalar]
            qn = [0]

            def dma(out_, in_):
                qs[qn[0] % 3].dma_start(out=out_, in_=in_)
                qn[0] += 1

            nc.scalar.dma_start(out=bqk_sb[:], in_=bqk.rearrange("(s p) -> p s", p=128))
            nc.sync.dma_start(out=ones_sb[:], in_=ones[:])
            for o in range(KO):
                # rotate each stream across queues per chunk so the 4MB xT
                # stream isn't serialized behind one DGE (~160GB/s/queue)
                qs[o % 3].dma_start(
                    out=wqk_ch[o][:], in_=wqk[o * 128 : (o + 1) * 128, :]
                )
                qs[(o + 1) % 3].dma_start(
                    out=xT_ch[o][:], in_=xT[o * 128 : (o + 1) * 128, :]
                )
                qs[(o + 2) % 3].dma_start(
                    out=wv_sb[:, o, :], in_=wv[o * 128 : (o + 1) * 128, :]
                )
            dma(wout_sb[:, 0, :], wout[0:128, :])
            dma(wout_sb[:, 1, :], wout[128:256, :])
            dma(bv_sb[:], bv[None, :])

            # --- startup projections, o-outer so the PE tracks the DMA
            # stream: slot0 n0 (q cols for item 0), slot1 n0-3 (all K^T for
            # pair 0), and NV0 V tiles accumulate in PSUM as chunks land.
            with (
                tc.tile_pool(name="psB", bufs=1, space="PSUM") as psB,
                tc.tile_pool(name="psC", bufs=1, space="PSUM") as psC,
            ):
                ps_s0 = psB.tile([128, QC], F32, tag="s0n0", name="ps_s0")
                ps_s1 = [
                    psB.tile([128, QC], F32, tag=f"s1n{n}", name=f"ps_s1{n}")
                    for n in range(NQ)
                ]
                ps_v = [
                    psC.tile([128, CV], F32, tag=f"v{j}", name=f"ps_v{j}")
                    for j in range(NV0)
                ]
                def s0_mm(o):
                    nc.tensor.matmul(
                        ps_s0[:],
                        wqk_ch[o][:, 0:128],
                        xT_ch[o][:, 0:QC],
                        start=(o == 0),
                        stop=(o == KO - 1),
                    )

                def s1_mm(n, o):
                    nc.tensor.matmul(
                        ps_s1[n][:],
                        wqk_ch[o][:, 128:256],
                        xT_ch[o][:, n * QC : (n + 1) * QC],
                        start=(o == 0),
                        stop=(o == KO - 1),
                    )

                def v_mm(j, o):
                    nc.tensor.matmul(
                        ps_v[j][:],
                        xT_ch[o][:, j * 128 : (j + 1) * 128],
                        wv_sb[:, o, :],
                        start=(o == 0),
                        stop=(zero_bias and o == KO - 1),
                    )

                def v_fin(j):
                    if not zero_bias:
                        nc.tensor.matmul(
                            ps_v[j][:], ones_sb[0:1, 0:128], bv_sb[0:1, :],
                            start=False, stop=True,
                        )
                    nc.vector.tensor_copy(out=v_sb[:, j, :], in_=ps_v[j][:])
                    if zero_bias:
                        nc.vector.memset(v_sb[:, j, 64 :: DK + 1], 1.0)

                for o in range(KO - 1):
                    s0_mm(o)
                    for n in range(NQ):
                        s1_mm(n, o)
                    for j in range(NV0):
                        v_mm(j, o)
                # last chunk: only the two matmuls that gate the first
                # scores go first, then their copybacks (ACT + DVE in
                # parallel); the rest streams behind the first exp.
                s0_mm(KO - 1)
                s1_mm(0, KO - 1)
                nc.scalar.activation(
                    qkT_sb[:, 0, 0:QC], ps_s0[:], Ident,
                    bias=bqk_sb[:, 0:1], scale=1.0,
                )
                nc.vector.tensor_scalar(
                    out=qkT_sb[:, 1, 0:QC], in0=ps_s1[0][:],
                    scalar1=bqk_sb[:, 1:2], scalar2=None, op0=ADD,
                )
                for j in range(NV0):
                    v_mm(j, KO - 1)
                    v_fin(j)
                for n in range(1, NQ):
                    s1_mm(n, KO - 1)
                    if n % 2:
                        nc.scalar.activation(
                            qkT_sb[:, 1, n * QC : (n + 1) * QC], ps_s1[n][:],
                            Ident, bias=bqk_sb[:, 1:2], scale=1.0,
                        )
                    else:
                        nc.vector.tensor_scalar(
                            out=qkT_sb[:, 1, n * QC : (n + 1) * QC],
                            in0=ps_s1[n][:],
                            scalar1=bqk_sb[:, 1:2], scalar2=None, op0=ADD,
                        )

            # --- attention: 8 items (pair, q-chunk); per k-tile the ACT exp
            # (~1.08us) bounds the loop, PE slack absorbs filler units.
            items = [(p2, qc) for p2 in range(2) for qc in range(NQ)]
            with (
                tc.tile_pool(name="pt", bufs=4) as ptp,
                tc.tile_pool(name="rcp", bufs=3) as rcp,
                tc.tile_pool(name="psST", bufs=2, space="PSUM") as psST,
                tc.tile_pool(name="psOT", bufs=1, space="PSUM") as psOT,
                tc.tile_pool(name="psL", bufs=2, space="PSUM") as psL,
                tc.tile_pool(name="ysb", bufs=4) as ysb,
            ):

                def emit_v(lt):
                    """V_aug k-tile lt = x @ w_v_aug (ones cols by memset when
                    biases are zero).  ~0.87us of PE."""
                    ps = psL.tile([128, CV], F32, tag="px", name="psv")
                    for o in range(KO):
                        nc.tensor.matmul(
                            ps[:],
                            xT_ch[o][:, lt * 128 : (lt + 1) * 128],
                            wv_sb[:, o, :],
                            start=(o == 0),
                            stop=(zero_bias and o == KO - 1),
                        )
                    if not zero_bias:
                        nc.tensor.matmul(
                            ps[:], ones_sb[0:1, 0:128], bv_sb[0:1, :],
                            start=False, stop=True,
                        )
                    nc.vector.tensor_copy(out=v_sb[:, lt, :], in_=ps[:])
                    if zero_bias:
                        nc.vector.memset(v_sb[:, lt, 64 :: DK + 1], 1.0)

                def emit_qk_chunk(s, n):
                    """One n-chunk (512 q cols) of qkT slot s.  8 matmuls of
                    512 whose per-o LDWEIGHTS hide under the previous matmul
                    (256-col splits expose every load).  ~1.76us of PE."""
                    c0 = n * QC
                    ps = psL.tile([128, QC], F32, tag="px", name="psqk")
                    for o in range(KO):
                        nc.tensor.matmul(
                            ps[:],
                            wqk_ch[o][:, s * 128 : (s + 1) * 128],
                            xT_ch[o][:, c0 : c0 + QC],
                            start=(o == 0),
                            stop=(o == KO - 1),
                        )
                    nc.vector.tensor_scalar(
                        out=qkT_sb[:, s, c0 : c0 + QC], in0=ps[:],
                        scalar1=bqk_sb[:, s : s + 1], scalar2=None, op0=ADD,
                    )

                def emit_out_unit(qc, ltl, n2, last=False):
                    """One [128l, 512] block of the out-projection: both
                    pair-halves accumulate in one PSUM tile, then copy + DMA.
                    ~0.45us of PE."""
                    lt = 4 * qc + ltl
                    ps = psL.tile([128, QC], F32, tag="px", name="psy")
                    for s in range(2):
                        nc.tensor.matmul(
                            ps[:],
                            oT_sb[:, s, lt * 128 : (lt + 1) * 128],
                            wout_sb[:, s, n2 * QC : (n2 + 1) * QC],
                            start=(s == 0),
                            stop=(s == 1),
                        )
                    yt = ysb.tile([128, QC], F32, tag="yt", name="yt")
                    if last or state.get("tail"):
                        # tail: copybacks ride the idle ACT queue so the
                        # normalize's DVE chain isn't queued behind them
                        nc.scalar.copy(out=yt[:], in_=ps[:])
                    else:
                        nc.vector.tensor_copy(out=yt[:], in_=ps[:])
                    [nc.gpsimd, nc.sync][(ltl + n2) % 2].dma_start(
                        out=y[lt * 128 : (lt + 1) * 128, n2 * QC : (n2 + 1) * QC],
                        in_=yt[:],
                    )

                def norm_copy(po):
                    """Stage O^T_aug to SBUF (frees the PSUM accumulator for
                    the next item) and compute the reciprocal of the P-rowsum
                    row on the DVE queue, async to PE.  The row transposes
                    into 32 partitions (16 elems/lane) so the iterative
                    reciprocal costs ~16 not ~512 lane-cycles, then
                    transposes back: tr2 row 65 = 1/rowsum."""
                    po_sb = rcp.tile([96, QC], F32, tag="po_sb", name="po_sb", bufs=4)
                    nc.vector.tensor_copy(out=po_sb[0:65, :], in_=po[:])
                    tr = rcp.tile([96, QC], F32, tag="tr", name="tr", bufs=2)
                    trR = rcp.tile([96, QC], F32, tag="trR", name="trR", bufs=2)
                    tr2 = rcp.tile([96, QC], F32, tag="tr2", name="tr2", bufs=2)
                    rrb = rcp.tile([96, QC], BF16, tag="rrb", name="rrb", bufs=2)
                    nc.vector.transpose(out=tr[64:96, :], in_=po_sb[64:96, :])
                    nc.vector.reciprocal(
                        out=trR[64:96, 0 :: 32], in_=tr[64:96, 0 :: 32]
                    )
                    nc.vector.transpose(out=tr2[64:96, :], in_=trR[64:96, :])
                    # bf16 row so the K=1 broadcast matmul streams at full
                    # rate (f32 moving data runs the PE at half speed)
                    nc.vector.tensor_copy(out=rrb[64:65, :], in_=tr2[64:65, :])
                    return po_sb, rrb

                def norm_finish(po_sb, rr, he, p2, qc, rb_pool=None, rb_tag="px"):
                    """Broadcast the reciprocal row across 64 partitions with
                    a K=1 matmul (ones.T @ row), then multiply.  PE cost
                    ~0.39us; runs as a filler inside the next item."""
                    pool = rb_pool if rb_pool is not None else psL
                    rbps = pool.tile([64, QC], F32, tag=rb_tag, name="rbps")
                    nc.tensor.matmul(
                        rbps[:], ones64[64:65, :], rr[64:65, :],
                        start=True, stop=True,
                    )
                    nc.vector.tensor_mul(
                        out=oT_sb[he * 64 : (he + 1) * 64, p2, qc * QC : (qc + 1) * QC],
                        in0=po_sb[0:64, :],
                        in1=rbps[:],
                    )

                def st_pair(sq, sk, qc, kt):
                    """S^T for both heads of the pair, written into the two
                    halves of one 2-bank PSUM tile so a single wide ACTIVATE
                    exps both."""
                    ps2 = psST.tile([128, 2, QC], F32, tag="st2", name="st2")
                    nc.tensor.matmul(
                        ps2[:, 0, :],
                        qkT_sb[0:64, sk, kt * 128 : (kt + 1) * 128],
                        qkT_sb[0:64, sq, qc * QC : (qc + 1) * QC],
                        start=True,
                        stop=True,
                    )
                    nc.tensor.matmul(
                        ps2[:, 1, :],
                        qkT_sb[64:128, sk, kt * 128 : (kt + 1) * 128],
                        qkT_sb[64:128, sq, qc * QC : (qc + 1) * QC],
                        start=True,
                        stop=True,
                    )
                    return ps2

                # filler queue: (cost_ns, force_at_gkt_or_None, fn).
                # deadlines: qkT q-cols for item i are first read by the sts
                # prefetch at gkt = 16*i - 1; k-cols n by st(kt=4n) at
                # gkt = base + 4n - 1.
                fillers = collections.deque()

                def add_qk(s, n, g):
                    fillers.append(
                        (1760.0, g, (lambda s=s, n=n: emit_qk_chunk(s, n)))
                    )

                add_qk(0, 1, 15)
                add_qk(0, 2, 31)
                add_qk(0, 3, 47)
                add_qk(3, 0, 63)
                add_qk(2, 0, 63)
                add_qk(3, 1, 66)
                add_qk(3, 2, 70)
                add_qk(3, 3, 74)
                add_qk(2, 1, 79)
                add_qk(2, 2, 95)
                add_qk(2, 3, 111)

                state = {"credit": 0.0, "v_done": NV0, "st_next": 0}

                # prefetch queue of S^T tiles in global kt order; pump() tops
                # it up to depth 2 before a big filler lump so the exp stream
                # never starves behind the lump.
                st_q = collections.deque()

                def st_push():
                    g = state["st_next"]
                    if g >= len(items) * LT:
                        return
                    i2, k2 = divmod(g, LT)
                    pp, _qq = items[i2]
                    st_q.append(st_pair(2 * pp, 2 * pp + 1, _qq, k2))
                    state["st_next"] = g + 1

                def pump(gkt):
                    c = state["credit"]
                    while fillers and fillers[0][1] is not None and fillers[0][1] <= gkt + 2:
                        cost, _, fn = fillers.popleft()
                        if cost >= 900.0 and len(st_q) < 2:
                            st_push()
                        fn()
                        c -= cost
                    c = max(c, -2000.0)
                    # at most ~one filler's worth per kt keeps credit-driven
                    # pops from bunching into multi-us lumps at item ends
                    budget = 900.0
                    while fillers and c >= fillers[0][0] and budget > 0.0:
                        cost, _, fn = fillers.popleft()
                        if cost >= 900.0 and len(st_q) < 2:
                            st_push()
                        fn()
                        c -= cost
                        budget -= cost
                    state["credit"] = min(c, 2000.0)

                SLACK = 210.0
                st_push()
                for idx, (p2, qc) in enumerate(items):
                    po_e = psOT.tile([65, QC], F32, tag="ote", name="ote")
                    po_o = psOT.tile([65, QC], F32, tag="oto", name="oto")
                    for kt in range(LT):
                        gkt = idx * LT + kt
                        ps2 = st_q.popleft()
                        pt2 = ptp.tile([128, 2, QC], BF16, tag="pt2", name="pt2")
                        nc.scalar.activation(pt2[:], ps2[:], Exp)
                        if not st_q:
                            st_push()
                        if idx == 0:
                            # stream remaining V tiles one k-tile ahead
                            while state["v_done"] <= min(kt + 1, LT - 1):
                                emit_v(state["v_done"])
                                state["v_done"] += 1
                        nc.tensor.matmul(
                            po_e[:],
                            v_sb[:, kt, (2 * p2) * 65 : (2 * p2) * 65 + 65],
                            pt2[:, 0, :],
                            start=(kt == 0),
                            stop=(kt == LT - 1),
                        )
                        nc.tensor.matmul(
                            po_o[:],
                            v_sb[:, kt, (2 * p2 + 1) * 65 : (2 * p2 + 1) * 65 + 65],
                            pt2[:, 1, :],
                            start=(kt == 0),
                            stop=(kt == LT - 1),
                        )
                        state["credit"] += SLACK
                        pump(gkt)
                    sb_e, rr_e = norm_copy(po_e)
                    sb_o, rr_o = norm_copy(po_o)
                    if idx + 1 == len(items):
                        # tail: overlap the last normalize's DVE chain with
                        # leftover fillers and the pair-0 halves of the final
                        # out-projection (independent of this normalize), so
                        # the PE stays busy and keeps its clock up.
                        state["tail"] = True
                        while fillers:
                            _, _, fn = fillers.popleft()
                            fn()
                        held = []
                        for u in range(4):
                            ltl, n2 = divmod(u, 2)
                            lt = 4 * 3 + ltl
                            pool, tag = (psL, "px") if u < 2 else (psST, "st2")
                            ps = pool.tile([128, QC], F32, tag=tag, name=f"tps{u}")
                            nc.tensor.matmul(
                                ps[:],
                                oT_sb[:, 0, lt * 128 : (lt + 1) * 128],
                                wout_sb[:, 0, n2 * QC : (n2 + 1) * QC],
                                start=True,
                                stop=False,
                            )
                            held.append((ps, ltl, n2))
                        norm_finish(sb_e, rr_e, 0, p2, qc, rb_pool=psOT, rb_tag="ote")
                        norm_finish(sb_o, rr_o, 1, p2, qc, rb_pool=psOT, rb_tag="oto")
                        for u, (ps, ltl, n2) in enumerate(held):
                            lt = 4 * 3 + ltl
                            nc.tensor.matmul(
                                ps[:],
                                oT_sb[:, 1, lt * 128 : (lt + 1) * 128],
                                wout_sb[:, 1, n2 * QC : (n2 + 1) * QC],
                                start=False,
                                stop=True,
                            )
                            yt = ysb.tile([128, QC], F32, tag="yt", name="yt")
                            if u % 2 == 1:
                                nc.scalar.copy(out=yt[:], in_=ps[:])
                            else:
                                nc.vector.tensor_copy(out=yt[:], in_=ps[:])
                            [nc.gpsimd, nc.sync][(ltl + n2) % 2].dma_start(
                                out=y[lt * 128 : (lt + 1) * 128, n2 * QC : (n2 + 1) * QC],
                                in_=yt[:],
                            )
                    else:
                        # run the PE half of the normalize as early fillers
                        # inside the next item (the DVE half is already on
                        # the queue), so the PE never blocks on it here.
                        g_norm = (idx + 1) * LT + 10
                        fillers.appendleft(
                            (430.0, g_norm,
                             (lambda sb=sb_o, rr=rr_o, p=p2, q=qc: norm_finish(sb, rr, 1, p, q)))
                        )
                        fillers.appendleft(
                            (430.0, g_norm,
                             (lambda sb=sb_e, rr=rr_e, p=p2, q=qc: norm_finish(sb, rr, 0, p, q)))
                        )
                    if 4 <= idx < 7:
                        oqc = idx - 4
                        for ltl in range(4):
                            for n2 in range(2):
                                # the last two qc2 units deliberately have no
                                # deadline: they surface in the tail drain,
                                # giving the PE real work (whose copybacks
                                # queue behind the normalize's DVE chain)
                                # while the final normalize runs.
                                g_emit = (idx + 2) * LT - 10
                                cost = 450.0
                                if oqc == 2 and ltl >= 1:
                                    # held for the tail drain (cost sentinel
                                    # keeps the credit scheduler off them)
                                    g_emit = None
                                    cost = 1e9
                                fillers.append(
                                    (cost, g_emit,
                                     (lambda q=oqc, l=ltl, n=n2: emit_out_unit(q, l, n)))
                                )

                # remaining half of the last q-chunk's out-projection
                # (lt 14,15; copybacks alternate DVE/ACT)
                u = 0
                for ltl in range(2, 4):
                    for n2 in range(2):
                        emit_out_unit(3, ltl, n2, last=(u % 2 == 1))
                        u += 1
    _drop_redundant_ldweights(nc)
    _split_excess_waits(nc)
    return nc


def make_in_maps(x, w_qkv, b_qkv, w_out):
    """Per-core input shards.  Core i: batch i//4, head group i%4 (4 heads).

    w_qk column order per core: slots of 128 = (pair0 q | pair0 k | pair1 q |
    pair1 k), each slot = [even head (64) | odd head (64)].  The 1/sqrt(dk)
    scale is folded into the q columns (and q bias entries).
    """
    in_maps = []
    for core in range(8):
        b, g = divmod(core, 4)
        heads = [4 * g + j for j in range(HG)]
        xT = np.ascontiguousarray(x[b].T)
        cols, bias = [], []
        for pair in range(2):
            for qk in range(2):
                for j in range(2):
                    h = heads[2 * pair + j]
                    base = h * 3 * DK + qk * DK
                    c = w_qkv[:, base : base + DK]
                    bb = b_qkv[base : base + DK]
                    if qk == 0:
                        c = c * (1.0 / np.sqrt(DK))
                        bb = bb * (1.0 / np.sqrt(DK))
                    cols.append(c)
                    bias.append(bb)
        wqk = np.ascontiguousarray(np.concatenate(cols, axis=1), dtype=np.float32)
        bqk = np.concatenate(bias).astype(np.float32)
        wv = np.zeros((D, CV), np.float32)
        bv = np.zeros((CV,), np.float32)
        for j, h in enumerate(heads):
            base = h * 3 * DK + 2 * DK
            wv[:, 65 * j : 65 * j + 64] = w_qkv[:, base : base + DK]
            bv[65 * j : 65 * j + 64] = b_qkv[base : base + DK]
            bv[65 * j + 64] = 1.0
        wo = np.ascontiguousarray(w_out[g * 256 : (g + 1) * 256, :], dtype=np.float32)
        bf = ml_dtypes.bfloat16
        in_maps.append(
            {
                "xT": xT.astype(bf),
                "wqk": wqk.astype(bf),
                "bqk": bqk,
                "wv": wv.astype(bf),
                "bv": bv.astype(bf),
                "wout": wo.astype(bf),
                "ones": np.ones((1, L), bf),
            }
        )
    return in_maps


def kernel(**inputs):
    x = np.asarray(inputs["x"], np.float32)
    w_qkv = np.asarray(inputs["w_qkv"], np.float32)
    b_qkv = np.asarray(inputs["b_qkv"], np.float32)
    w_out = np.asarray(inputs["w_out"], np.float32)
    b_out = np.asarray(inputs["b_out"], np.float32)

    in_maps = make_in_maps(x, w_qkv, b_qkv, w_out)
    nc = build_nc(zero_bias=not bool(np.any(b_qkv)))
    res = run_bass_kernel_spmd(nc, in_maps, core_ids=list(range(8)))
    kernel.last_results = res

    out = np.zeros((B, L, D), np.float32)
    for core in range(8):
        out[core // 4] += res.results[core]["out"]
    out += b_out[None, None, :]
    return out


kernel.last_results = None
